# revision 27
# baseline (speedup 1.0000x reference)
"""GNN message-passing block on 8 Trainium2 NeuronCores.

Math: out[n] = relu(x_v[n] + agg_v[n] + agg_c[n])
    agg_v = segment_sum(MLPv(x_v)[src_vv], dst_vv)   (messages depend on src only)
    agg_c = Count @ MLPc(x_c)          (256 colors -> dense count matmul)

Design (v2, bf16):
  * Kernel A (node-sharded): computes the 50k-row message table in bf16.
  * Kernel B (dst-sharded): per-edge gather of bf16 pair-rows (256 B each,
    the dma_gather minimum element) + scatter-add via one-hot matmuls.
  * Edges are bucketed by 512-node dst range (one PSUM bank per bucket),
    split by src parity (a chunk's matmul reads the correct 64-column half
    of the gathered pair), and dst-sorted so each 128-edge chunk only
    covers a narrow 64-node window at a COMPILE-TIME offset (the window
    ladder w_c = 16c - 5.6*sqrt(c) is feasible w.h.p. for uniform edges;
    the host greedy verifies and bumps the chunk count on failure).
    The two full-width color-count matmuls run first with start=True so
    every PSUM element is initialized regardless of window coverage gaps.
  * The 13 per-bucket gathers rotate across all 4 SWDGE queues with deep
    buffering so descriptor generation and SDMA drain never go idle.
"""

import math

import numpy as np

import concourse.bacc as bacc
import concourse.mybir as mybir
import concourse.tile as tile
from concourse import ap_utils
from concourse._compat import exact_div
from concourse.bass import MemorySpace
from concourse.bass_utils import run_bass_kernel_spmd

FP32 = mybir.dt.float32
BF16 = mybir.dt.bfloat16
I16 = mybir.dt.int16
AF = mybir.ActivationFunctionType
NPBF16 = mybir.dt.np(BF16)

N_CORES = 8
N_NODES = 50000
N_COLORS = 256
D = 64
H = 128
NP = 50176              # nodes padded to 392 tiles of 128
PC = NP // N_CORES      # 6272 nodes per core
BK = 512                # bucket = one PSUM bank of fp32
NB_FULL = PC // BK      # 12 full buckets; tail bucket of 128 nodes
TAIL = PC - NB_FULL * BK
PAIRS = NP // 2 + 128   # bf16 pair-row table rows (padded)
WIN = 64                # one-hot window width
PAD_DST = 100.0

PROFILE = False
LAST_EXEC_NS = {}
_TRIM_TAIL = True

_cache = {}


def _run(nc, in_maps, label):
    kwargs = {}
    if PROFILE:
        kwargs = dict(trace=True, trace_cores=[0])
    try:
        res = run_bass_kernel_spmd(nc, in_maps, list(range(N_CORES)), **kwargs)
    except Exception:
        if not kwargs:
            raise
        res = run_bass_kernel_spmd(nc, in_maps, list(range(N_CORES)))
    LAST_EXEC_NS[label] = res.exec_time_ns
    return res.results


def _dma_gather128(eng, out_ap, in_ap, idxs_ap, num_idxs, num_idxs_reg,
                   elem_size, elem_step, queue_num):
    """bass dma_gather for 128-byte elements.

    Identical to bass.GpSimd.dma_gather (non-transpose, DRAM source,
    immediate trigger) except the element only has to be a multiple of
    128 B; the row stride must still be a multiple of 256 B, which is the
    only granularity the descriptor ucode actually requires
    (stride_bytes_256).  The ucode's non-transpose path emits one plain
    CME descriptor of elem_size bytes per index, so 128 B is fine.
    """
    eng._assert_queue_num(queue_num)
    assert idxs_ap.dtype == mybir.dt.int16
    assert in_ap.dtype == out_ap.dtype
    assert in_ap.space == MemorySpace.DRAM
    assert idxs_ap.space == MemorySpace.SBUF
    assert out_ap.space == MemorySpace.SBUF
    elem_size_bytes = elem_size * mybir.dt.size(in_ap.dtype)
    assert elem_size_bytes % 128 == 0
    assert ap_utils.ap_is_contiguous(in_ap.ap[1:])
    assert ap_utils.ap_is_contiguous(out_ap.ap[1:])
    assert ap_utils.ap_is_contiguous(idxs_ap.ap[1:])
    assert in_ap.ap[-1][1] == out_ap.ap[-1][1] == elem_size
    assert out_ap.ap[0][1] * out_ap.ap[1][1] == num_idxs
    assert in_ap.ap[0][0] == elem_step
    stride_bytes_256 = exact_div(elem_step * mybir.dt.size(in_ap.dtype), 256)
    assert stride_bytes_256 < 256
    return eng.add_instruction(
        mybir.InstDMAGatherAnt(
            name=eng.bass.get_next_instruction_name(),
            ins=[
                *eng.lower_ap_dma(in_ap, for_custom_bir_dma=True),
                eng.lower_ap(idxs_ap),
                eng.lower_val_access(eng.to_reg(num_idxs_reg)),
            ],
            outs=[eng.lower_ap(out_ap)],
            transpose=False,
            num_idxs=num_idxs,
            elem_size=elem_size,
            stride_bytes_256=stride_bytes_256,
            gen_mode=0,
            single_packet=False,
            queue_num=queue_num,
            sbuf_tokens_per_rank=0,
            sbuf_free_dim_per_rank=0,
            sbuf_free_dim_pad_per_rank=0,
            sbuf_byte_offset=0,
        )
    )


def _windows(n_chunks, width):
    """Compile-time window offsets; clamped ascending ladder."""
    top = width - WIN
    ws = []
    for c in range(n_chunks):
        w = int(round(16 * c - 5.6 * math.sqrt(c)))
        ws.append(min(top, max(0, w)))
    return ws


# ---------------------------------------------------------------- kernel A
def _build_kernel_a():
    if "A" in _cache:
        return _cache["A"]
    nc = bacc.Bacc("TRN2", target_bir_lowering=False, debug=False,
                   num_devices=N_CORES)
    xT = nc.dram_tensor("xT", [D, PC], BF16, kind="ExternalInput")
    w1 = nc.dram_tensor("w1", [D, H], BF16, kind="ExternalInput")
    b1 = nc.dram_tensor("b1", [H, 1], FP32, kind="ExternalInput")
    w2 = nc.dram_tensor("w2", [H, D], BF16, kind="ExternalInput")
    b2 = nc.dram_tensor("b2", [D, 1], FP32, kind="ExternalInput")
    xcT = nc.dram_tensor("xcT", [D, N_COLORS], BF16, kind="ExternalInput")
    w1c = nc.dram_tensor("w1c", [D, H], BF16, kind="ExternalInput")
    b1c = nc.dram_tensor("b1c", [H, 1], FP32, kind="ExternalInput")
    w2c = nc.dram_tensor("w2c", [H, D], BF16, kind="ExternalInput")
    b2c = nc.dram_tensor("b2c", [D, 1], FP32, kind="ExternalInput")
    msgT = nc.dram_tensor("msgT", [D, PC], BF16, kind="ExternalOutput")
    msgcT = nc.dram_tensor("msgcT", [D, N_COLORS], BF16, kind="ExternalOutput")

    S = 512
    with tile.TileContext(nc) as tc:
        with (
            tc.tile_pool(name="w", bufs=1) as wp,
            tc.tile_pool(name="act", bufs=4) as ap,
            tc.tile_pool(name="ps", bufs=4, space="PSUM") as pp,
        ):
            def mlp(xT_d, w1_d, b1_d, w2_d, b2_d, out_d, n_cols, tag):
                w1_t = wp.tile([D, H], BF16, tag=f"w1{tag}")
                b1_t = wp.tile([H, 1], FP32, tag=f"b1{tag}")
                w2_t = wp.tile([H, D], BF16, tag=f"w2{tag}")
                b2_t = wp.tile([D, 1], FP32, tag=f"b2{tag}")
                nc.sync.dma_start(out=w1_t[:], in_=w1_d[:])
                nc.sync.dma_start(out=b1_t[:], in_=b1_d[:])
                nc.sync.dma_start(out=w2_t[:], in_=w2_d[:])
                nc.sync.dma_start(out=b2_t[:], in_=b2_d[:])
                for s0 in range(0, n_cols, S):
                    s1 = min(s0 + S, n_cols)
                    w = s1 - s0
                    x_t = ap.tile([D, S], BF16, tag="x")
                    nc.sync.dma_start(out=x_t[:, :w], in_=xT_d[:, s0:s1])
                    h_ps = pp.tile([H, S], FP32, tag="h")
                    nc.tensor.matmul(out=h_ps[:, :w], lhsT=w1_t[:],
                                     rhs=x_t[:, :w], start=True, stop=True)
                    h_sb = ap.tile([H, S], BF16, tag="h_sb")
                    nc.scalar.activation(h_sb[:, :w], h_ps[:, :w], AF.Relu,
                                         bias=b1_t[:])
                    m_ps = pp.tile([D, S], FP32, tag="m")
                    nc.tensor.matmul(out=m_ps[:, :w], lhsT=w2_t[:],
                                     rhs=h_sb[:, :w], start=True, stop=True)
                    m_sb = ap.tile([D, S], BF16, tag="m_sb")
                    nc.scalar.activation(m_sb[:, :w], m_ps[:, :w], AF.Identity,
                                         bias=b2_t[:])
                    nc.sync.dma_start(out=out_d[:, s0:s1], in_=m_sb[:, :w])

            mlp(xT, w1, b1, w2, b2, msgT, PC, "v")
            mlp(xcT, w1c, b1c, w2c, b2c, msgcT, N_COLORS, "c")
    nc.compile()
    _cache["A"] = nc
    return nc


# ---------------------------------------------------------------- kernel B
def _build_kernel_b(CE, CEt, Ks):
    key = ("B", CE, CEt, Ks)
    if key in _cache:
        return _cache[key]
    CB = 2 * CE            # chunk columns per full bucket (even + odd)
    CBt = 2 * CEt
    IDXF = NB_FULL * CB * 8 + CBt * 8
    DLC = NB_FULL * CB + CBt

    nc = bacc.Bacc("TRN2", target_bir_lowering=False, debug=False,
                   num_devices=N_CORES, num_swdge_queues=4)
    table = nc.dram_tensor("table", [PAIRS, 2 * D], BF16, kind="ExternalInput")
    msgc = nc.dram_tensor("msgc", [N_COLORS, D], BF16, kind="ExternalInput")
    countT = nc.dram_tensor("countT", [N_COLORS, PC], BF16,
                            kind="ExternalInput")
    xT = nc.dram_tensor("xT", [D, PC], FP32, kind="ExternalInput")
    iota = nc.dram_tensor("iota", [128, WIN * CB], BF16, kind="ExternalInput")
    iota_tl = nc.dram_tensor("iota_tl", [128, WIN * CBt], BF16,
                             kind="ExternalInput")
    dstloc = nc.dram_tensor("dstloc", [128, DLC], BF16, kind="ExternalInput")
    gidx = nc.dram_tensor("gidx", [128, IDXF], I16, kind="ExternalInput")
    outT = nc.dram_tensor("outT", [D, PC], FP32, kind="ExternalOutput")

    wins_full = _windows(CE, BK)
    wins_tail = _windows(CEt, TAIL)

    with tile.TileContext(nc) as tc:
        with (
            tc.tile_pool(name="const", bufs=1) as cp,
            tc.tile_pool(name="gath", bufs=5) as gp,
            tc.tile_pool(name="idx", bufs=NB_FULL + 1) as ip,
            tc.tile_pool(name="oh", bufs=3) as ohp,
            tc.tile_pool(name="ct", bufs=4) as ctp,
            tc.tile_pool(name="ep", bufs=3) as ep,
            tc.tile_pool(name="ps", bufs=4, space="PSUM") as pp,
        ):
            # index tiles first so the first gathers launch immediately
            idx_es, idx_os = [], []
            for b in range(NB_FULL + 1):
                C1 = CE if b < NB_FULL else CEt
                ixoff = b * CB * 8
                idx_e = ip.tile([128, C1 * 8], I16, tag="ide")
                idx_o = ip.tile([128, C1 * 8], I16, tag="ido")
                nc.sync.dma_start(out=idx_e[:],
                                  in_=gidx[:, ixoff:ixoff + C1 * 8])
                nc.sync.dma_start(out=idx_o[:],
                                  in_=gidx[:, ixoff + C1 * 8:ixoff + 2 * C1 * 8])
                idx_es.append(idx_e)
                idx_os.append(idx_o)

            iota_t = cp.tile([128, WIN * CB], BF16, tag="iota")
            nc.scalar.dma_start(out=iota_t[:], in_=iota[:])
            iota_tt = cp.tile([128, WIN * CBt], BF16, tag="iota_tl")
            nc.scalar.dma_start(out=iota_tt[:], in_=iota_tl[:])
            dl_t = cp.tile([128, DLC], BF16, tag="dstloc")
            nc.scalar.dma_start(out=dl_t[:], in_=dstloc[:])
            mc0 = cp.tile([128, D], BF16, tag="msgc0")
            mc1 = cp.tile([128, D], BF16, tag="msgc1")
            nc.scalar.dma_start(out=mc0[:], in_=msgc[0:128, :])
            nc.scalar.dma_start(out=mc1[:], in_=msgc[128:256, :])

            for b in range(NB_FULL + 1):
                full = b < NB_FULL
                W = BK if full else TAIL
                C1 = CE if full else CEt
                wins = wins_full if full else wins_tail
                n0 = b * BK
                dloff = b * CB
                it_t = iota_t if full else iota_tt

                ge = gp.tile([128, C1, D], BF16, tag="ge")
                go = gp.tile([128, C1, D], BF16, tag="go")
                # slots >= K carry idx -1 on every core: the gather ucode
                # trims them (fewer descriptors) and never writes them, so
                # zero that tail region first.
                for g_t, K in ((ge, Ks[b][0]), (go, Ks[b][1])):
                    c0 = K // 128
                    if c0 < C1:
                        # whole chunks: the gather rewrites slots < K after
                        nc.vector.memset(g_t[:, c0:C1, :], 0.0)
                _dma_gather128(nc.gpsimd, ge[:], table[:, 0:D], idx_es[b][:],
                               C1 * 128, Ks[b][0], D, elem_step=2 * D,
                               queue_num=(2 * b) % 4)
                _dma_gather128(nc.gpsimd, go[:], table[:, D:2 * D],
                               idx_os[b][:], C1 * 128, Ks[b][1], D,
                               elem_step=2 * D, queue_num=(2 * b + 1) % 4)

                oh_t = ohp.tile([128, WIN, 2 * C1], BF16, tag="oh")
                nc.vector.tensor_tensor(
                    out=oh_t[:],
                    in0=it_t[:, :WIN * 2 * C1].rearrange(
                        "p (n c) -> p n c", c=2 * C1),
                    in1=dl_t[:, dloff:dloff + 2 * C1].rearrange(
                        "p (o c) -> p o c", o=1).to_broadcast(
                            [128, WIN, 2 * C1]),
                    op=mybir.AluOpType.is_equal)

                ct0 = ctp.tile([128, W], BF16, tag="ct0")
                ct1 = ctp.tile([128, W], BF16, tag="ct1")
                nc.sync.dma_start(out=ct0[:], in_=countT[0:128, n0:n0 + W])
                nc.sync.dma_start(out=ct1[:], in_=countT[128:256, n0:n0 + W])

                ps = pp.tile([D, W], FP32, tag="agg")
                nc.tensor.matmul(out=ps[:], lhsT=mc0[:], rhs=ct0[:],
                                 start=True, stop=False)
                nc.tensor.matmul(out=ps[:], lhsT=mc1[:], rhs=ct1[:],
                                 start=False, stop=False)
                for c in range(C1):      # even-parity chunks
                    w0 = wins[c]
                    nc.tensor.matmul(out=ps[:, w0:w0 + WIN],
                                     lhsT=ge[:, c, :],
                                     rhs=oh_t[:, :, c],
                                     start=False, stop=False)
                for c in range(C1):      # odd-parity chunks
                    w0 = wins[c]
                    nc.tensor.matmul(out=ps[:, w0:w0 + WIN],
                                     lhsT=go[:, c, :],
                                     rhs=oh_t[:, :, C1 + c],
                                     start=False, stop=(c == C1 - 1))

                xa = ep.tile([D, W], FP32, tag="xa")
                nc.sync.dma_start(out=xa[:], in_=xT[:, n0:n0 + W])
                s_t = ep.tile([D, W], FP32, tag="s")
                nc.vector.tensor_tensor(out=s_t[:], in0=ps[:], in1=xa[:],
                                        op=mybir.AluOpType.add)
                o_t = ep.tile([D, W], FP32, tag="o")
                nc.scalar.activation(o_t[:], s_t[:], AF.Relu)
                nc.sync.dma_start(out=outT[:, n0:n0 + W], in_=o_t[:])
    nc.compile()
    _cache[key] = nc
    return nc


# ------------------------------------------------------------- host logic
def _wrap_idx(flat):
    """dma_gather index layout: [16, n/16] wrapped, replicated to 128 rows."""
    n = flat.shape[0]
    assert n % 16 == 0
    w = flat.reshape(n // 16, 16).T.astype(np.int16)
    return np.tile(w, (8, 1))


def _assign_bucket(d_arr, pidx_arr, width, n_chunks, wins):
    """Greedy window assignment for one (core,bucket,parity) edge group.

    d_arr must be sorted ascending. Returns (slot_idx [C,128] int64,
    slot_dst [C,128] float32) or None if infeasible.
    """
    slot_idx = np.zeros((n_chunks, 128), np.int64)
    slot_dst = np.full((n_chunks, 128), PAD_DST, np.float32)
    if d_arr.shape[0] == 0:
        return slot_idx, slot_dst
    wins_a = np.asarray(wins)
    # lo[d]: first chunk whose window contains d; hi[d]: last such chunk
    ds = np.arange(width)
    lo_map = np.searchsorted(wins_a, ds - (WIN - 1), side="left")
    hi_map = np.searchsorted(wins_a, ds, side="right") - 1
    cnt = np.bincount(d_arr, minlength=width)
    fills = np.zeros(n_chunks, np.int64)
    pos = 0
    for d in range(width):
        need = int(cnt[d])
        if need == 0:
            continue
        c = int(lo_map[d])
        hi = int(hi_map[d])
        while need > 0:
            if c > hi or c >= n_chunks:
                return None
            take = min(need, 128 - int(fills[c]))
            if take > 0:
                f = int(fills[c])
                slot_idx[c, f:f + take] = pidx_arr[pos:pos + take]
                slot_dst[c, f:f + take] = d - wins[c]
                fills[c] += take
                pos += take
                need -= take
            if need > 0:
                c += 1
    return slot_idx, slot_dst


def _used_count(slot_dst):
    """Number of slots up to and including the last real edge (flat order)."""
    used = slot_dst.reshape(-1) != PAD_DST
    nz = np.nonzero(used)[0]
    return int(nz[-1]) + 1 if nz.size else 0


def _prep_vv(src, dst):
    """Bucket/sort/pad vv edges; returns CE, CEt, per-core gidx and dstloc."""
    src = src.astype(np.int64)
    dst = dst.astype(np.int64)
    core = dst // PC
    d_in_core = dst - core * PC
    bucket = np.minimum(d_in_core // BK, NB_FULL)
    d_local = d_in_core - bucket * BK
    parity = src & 1
    pidx = src >> 1

    key = ((core * (NB_FULL + 1) + bucket) * 2 + parity)
    order = np.lexsort((d_local, key))
    key_s = key[order]
    d_s = d_local[order]
    p_s = pidx[order]
    n_groups = N_CORES * (NB_FULL + 1) * 2
    counts = np.bincount(key_s, minlength=n_groups)
    starts = np.concatenate([[0], np.cumsum(counts)[:-1]])

    # global chunk counts
    cnt_full = counts.reshape(N_CORES, NB_FULL + 1, 2)
    CE = max(1, int(np.ceil(cnt_full[:, :NB_FULL, :].max() / 128)))
    CEt = max(1, int(np.ceil(cnt_full[:, NB_FULL, :].max() / 128)))

    for _ in range(4):
        wins_full = _windows(CE, BK)
        wins_tail = _windows(CEt, TAIL)
        res = [[None] * (2 * (NB_FULL + 1)) for _ in range(N_CORES)]
        ok = True
        for k in range(N_CORES):
            for b in range(NB_FULL + 1):
                fullb = b < NB_FULL
                width = BK if fullb else TAIL
                C1 = CE if fullb else CEt
                wins = wins_full if fullb else wins_tail
                for par in range(2):
                    gk = (k * (NB_FULL + 1) + b) * 2 + par
                    s0, c0 = starts[gk], counts[gk]
                    r = _assign_bucket(d_s[s0:s0 + c0], p_s[s0:s0 + c0],
                                       width, C1, wins)
                    if r is None:
                        ok = False
                        break
                    res[k][b * 2 + par] = r
                if not ok:
                    break
            if not ok:
                break
        if ok:
            break
        CE += 1
        CEt += 1
    else:
        raise RuntimeError("window assignment infeasible")

    # core-uniform trim counts: the gather ucode trims trailing -1 indices
    # and the decode reserves ring space from num_idxs_reg, so the trimmed
    # count must be identical on every core.
    Ks = []
    for b in range(NB_FULL + 1):
        kpair = []
        for par in range(2):
            n = max(_used_count(res[k][b * 2 + par][1])
                    for k in range(N_CORES))
            if not _TRIM_TAIL:
                n = (CE if b < NB_FULL else CEt) * 128
            kpair.append(max(n, 128))
        Ks.append(tuple(kpair))
    Ks = tuple(Ks)

    gidx, dstloc = [], []
    for k in range(N_CORES):
        parts_i, parts_d = [], []
        for b in range(NB_FULL + 1):
            ie, de = res[k][b * 2 + 0]
            io, do = res[k][b * 2 + 1]
            fe = ie.reshape(-1).copy()
            fo = io.reshape(-1).copy()
            fe[Ks[b][0]:] = -1
            fo[Ks[b][1]:] = -1
            parts_i.append(np.concatenate(
                [_wrap_idx(fe), _wrap_idx(fo)], axis=1))
            dl = np.concatenate([de, do], axis=0).T    # [128, 2*C1]
            parts_d.append(np.ascontiguousarray(dl))
        gidx.append(np.concatenate(parts_i, axis=1))
        dstloc.append(np.concatenate(parts_d, axis=1).astype(NPBF16))
    return CE, CEt, gidx, dstloc, Ks


def kernel(x_v, x_c, W1v, b1v, W2v, b2v, W1c, b1c, W2c, b2c,
           src_vv, dst_vv, src_vc, dst_vc):
    x_v = np.asarray(x_v, np.float32)
    x_c = np.asarray(x_c, np.float32)
    src_vv = np.asarray(src_vv, np.int32)
    dst_vv = np.asarray(dst_vv, np.int32)
    src_vc = np.asarray(src_vc, np.int32)
    dst_vc = np.asarray(dst_vc, np.int32)

    # ---------------- kernel A: message tables ----------------
    xT_full = np.zeros((D, NP), np.float32)
    xT_full[:, :N_NODES] = x_v.T
    a_common = {
        "w1": np.asarray(W1v, np.float32).astype(NPBF16),
        "b1": np.asarray(b1v, np.float32).reshape(H, 1),
        "w2": np.asarray(W2v, np.float32).astype(NPBF16),
        "b2": np.asarray(b2v, np.float32).reshape(D, 1),
        "xcT": np.ascontiguousarray(x_c.T).astype(NPBF16),
        "w1c": np.asarray(W1c, np.float32).astype(NPBF16),
        "b1c": np.asarray(b1c, np.float32).reshape(H, 1),
        "w2c": np.asarray(W2c, np.float32).astype(NPBF16),
        "b2c": np.asarray(b2c, np.float32).reshape(D, 1),
    }
    in_maps_a = []
    for k in range(N_CORES):
        m = dict(a_common)
        m["xT"] = np.ascontiguousarray(
            xT_full[:, k * PC:(k + 1) * PC]).astype(NPBF16)
        in_maps_a.append(m)
    nc_a = _build_kernel_a()
    res_a = _run(nc_a, in_maps_a, "A")

    msg = np.concatenate(
        [np.asarray(res_a[k]["msgT"]) for k in range(N_CORES)], axis=1).T
    msg_c = np.ascontiguousarray(np.asarray(res_a[0]["msgcT"]).T)  # [256,64]

    table = np.zeros((PAIRS, 2 * D), NPBF16)
    table[:NP // 2] = msg.reshape(NP // 2, 2 * D)

    # ---------------- host: index prep ----------------
    CE, CEt, gidx, dstloc, Ks = _prep_vv(src_vv, dst_vv)

    cnt = np.bincount(src_vc.astype(np.int64) * NP + dst_vc,
                      minlength=N_COLORS * NP).reshape(N_COLORS, NP)
    countT = cnt.astype(NPBF16)

    CB = 2 * CE
    CBt = 2 * CEt
    iota = np.tile(np.repeat(np.arange(WIN, dtype=np.float32), CB),
                   (128, 1)).astype(NPBF16)
    iota_tl = np.tile(np.repeat(np.arange(WIN, dtype=np.float32), CBt),
                      (128, 1)).astype(NPBF16)

    # ---------------- kernel B: gather + scatter + epilogue ----------------
    in_maps_b = []
    for k in range(N_CORES):
        in_maps_b.append({
            "table": table,
            "msgc": np.ascontiguousarray(msg_c.astype(NPBF16)),
            "countT": np.ascontiguousarray(countT[:, k * PC:(k + 1) * PC]),
            "xT": np.ascontiguousarray(xT_full[:, k * PC:(k + 1) * PC]),
            "iota": iota,
            "iota_tl": iota_tl,
            "dstloc": dstloc[k],
            "gidx": gidx[k],
        })
    nc_b = _build_kernel_b(CE, CEt, Ks)
    res_b = _run(nc_b, in_maps_b, "B")

    outT = np.concatenate(
        [np.asarray(res_b[k]["outT"]) for k in range(N_CORES)], axis=1)
    return np.ascontiguousarray(outT.T[:N_NODES]).astype(np.float32)


# revision 30
# speedup vs baseline: 1.0123x; 1.0123x over previous
"""GNN message-passing block on 8 Trainium2 NeuronCores.

Math: out[n] = relu(x_v[n] + agg_v[n] + agg_c[n])
    agg_v = segment_sum(MLPv(x_v)[src_vv], dst_vv)   (messages depend on src only)
    agg_c = Count @ MLPc(x_c)          (256 colors -> dense count matmul)

Design (v2, bf16):
  * Kernel A (node-sharded): computes the 50k-row message table in bf16.
  * Kernel B (dst-sharded): per-edge gather of bf16 pair-rows (256 B each,
    the dma_gather minimum element) + scatter-add via one-hot matmuls.
  * Edges are bucketed by 512-node dst range (one PSUM bank per bucket),
    split by src parity (a chunk's matmul reads the correct 64-column half
    of the gathered pair), and dst-sorted so each 128-edge chunk only
    covers a narrow 64-node window at a COMPILE-TIME offset (the window
    ladder w_c = 16c - 5.6*sqrt(c) is feasible w.h.p. for uniform edges;
    the host greedy verifies and bumps the chunk count on failure).
    The two full-width color-count matmuls run first with start=True so
    every PSUM element is initialized regardless of window coverage gaps.
  * The 13 per-bucket gathers rotate across all 4 SWDGE queues with deep
    buffering so descriptor generation and SDMA drain never go idle.
"""

import math

import numpy as np

import concourse.bacc as bacc
import concourse.mybir as mybir
import concourse.tile as tile
from concourse import ap_utils
from concourse._compat import exact_div
from concourse.bass import MemorySpace
from concourse.bass_utils import run_bass_kernel_spmd

FP32 = mybir.dt.float32
BF16 = mybir.dt.bfloat16
I16 = mybir.dt.int16
AF = mybir.ActivationFunctionType
NPBF16 = mybir.dt.np(BF16)

N_CORES = 8
N_NODES = 50000
N_COLORS = 256
D = 64
H = 128
NP = 50176              # nodes padded to 392 tiles of 128
PC = NP // N_CORES      # 6272 nodes per core
BK = 512                # bucket = one PSUM bank of fp32
NB_FULL = PC // BK      # 12 full buckets; tail bucket of 128 nodes
TAIL = PC - NB_FULL * BK
PAIRS = NP // 2 + 128   # bf16 pair-row table rows (padded)
WIN = 64                # one-hot window width
PAD_DST = 100.0

PROFILE = False
LAST_EXEC_NS = {}
_TRIM_TAIL = True

_cache = {}


def _run(nc, in_maps, label):
    kwargs = {}
    if PROFILE:
        kwargs = dict(trace=True, trace_cores=[0])
    try:
        res = run_bass_kernel_spmd(nc, in_maps, list(range(N_CORES)), **kwargs)
    except Exception:
        if not kwargs:
            raise
        res = run_bass_kernel_spmd(nc, in_maps, list(range(N_CORES)))
    LAST_EXEC_NS[label] = res.exec_time_ns
    return res.results


def _dma_gather128(eng, out_ap, in_ap, idxs_ap, num_idxs, num_idxs_reg,
                   elem_size, elem_step, queue_num):
    """bass dma_gather for 128-byte elements.

    Identical to bass.GpSimd.dma_gather (non-transpose, DRAM source,
    immediate trigger) except the element only has to be a multiple of
    128 B; the row stride must still be a multiple of 256 B, which is the
    only granularity the descriptor ucode actually requires
    (stride_bytes_256).  The ucode's non-transpose path emits one plain
    CME descriptor of elem_size bytes per index, so 128 B is fine.
    """
    eng._assert_queue_num(queue_num)
    assert idxs_ap.dtype == mybir.dt.int16
    assert in_ap.dtype == out_ap.dtype
    assert in_ap.space == MemorySpace.DRAM
    assert idxs_ap.space == MemorySpace.SBUF
    assert out_ap.space == MemorySpace.SBUF
    elem_size_bytes = elem_size * mybir.dt.size(in_ap.dtype)
    assert elem_size_bytes % 128 == 0
    assert ap_utils.ap_is_contiguous(in_ap.ap[1:])
    assert ap_utils.ap_is_contiguous(out_ap.ap[1:])
    assert ap_utils.ap_is_contiguous(idxs_ap.ap[1:])
    assert in_ap.ap[-1][1] == out_ap.ap[-1][1] == elem_size
    assert out_ap.ap[0][1] * out_ap.ap[1][1] == num_idxs
    assert in_ap.ap[0][0] == elem_step
    stride_bytes_256 = exact_div(elem_step * mybir.dt.size(in_ap.dtype), 256)
    assert stride_bytes_256 < 256
    return eng.add_instruction(
        mybir.InstDMAGatherAnt(
            name=eng.bass.get_next_instruction_name(),
            ins=[
                *eng.lower_ap_dma(in_ap, for_custom_bir_dma=True),
                eng.lower_ap(idxs_ap),
                eng.lower_val_access(eng.to_reg(num_idxs_reg)),
            ],
            outs=[eng.lower_ap(out_ap)],
            transpose=False,
            num_idxs=num_idxs,
            elem_size=elem_size,
            stride_bytes_256=stride_bytes_256,
            gen_mode=0,
            single_packet=False,
            queue_num=queue_num,
            sbuf_tokens_per_rank=0,
            sbuf_free_dim_per_rank=0,
            sbuf_free_dim_pad_per_rank=0,
            sbuf_byte_offset=0,
        )
    )


def _windows(n_chunks, width):
    """Compile-time window offsets; clamped ascending ladder."""
    top = width - WIN
    ws = []
    for c in range(n_chunks):
        w = int(round(16 * c - 5.6 * math.sqrt(c)))
        ws.append(min(top, max(0, w)))
    return ws


# ---------------------------------------------------------------- kernel A
def _build_kernel_a():
    if "A" in _cache:
        return _cache["A"]
    nc = bacc.Bacc("TRN2", target_bir_lowering=False, debug=False,
                   num_devices=N_CORES)
    xT = nc.dram_tensor("xT", [D, PC], BF16, kind="ExternalInput")
    w1 = nc.dram_tensor("w1", [D, H], BF16, kind="ExternalInput")
    b1 = nc.dram_tensor("b1", [H, 1], FP32, kind="ExternalInput")
    w2 = nc.dram_tensor("w2", [H, D], BF16, kind="ExternalInput")
    b2 = nc.dram_tensor("b2", [D, 1], FP32, kind="ExternalInput")
    xcT = nc.dram_tensor("xcT", [D, N_COLORS], BF16, kind="ExternalInput")
    w1c = nc.dram_tensor("w1c", [D, H], BF16, kind="ExternalInput")
    b1c = nc.dram_tensor("b1c", [H, 1], FP32, kind="ExternalInput")
    w2c = nc.dram_tensor("w2c", [H, D], BF16, kind="ExternalInput")
    b2c = nc.dram_tensor("b2c", [D, 1], FP32, kind="ExternalInput")
    msgT = nc.dram_tensor("msgT", [D, PC], BF16, kind="ExternalOutput")
    msgcT = nc.dram_tensor("msgcT", [D, N_COLORS], BF16, kind="ExternalOutput")

    S = 512
    with tile.TileContext(nc) as tc:
        with (
            tc.tile_pool(name="w", bufs=1) as wp,
            tc.tile_pool(name="act", bufs=4) as ap,
            tc.tile_pool(name="ps", bufs=4, space="PSUM") as pp,
        ):
            def mlp(xT_d, w1_d, b1_d, w2_d, b2_d, out_d, n_cols, tag):
                w1_t = wp.tile([D, H], BF16, tag=f"w1{tag}")
                b1_t = wp.tile([H, 1], FP32, tag=f"b1{tag}")
                w2_t = wp.tile([H, D], BF16, tag=f"w2{tag}")
                b2_t = wp.tile([D, 1], FP32, tag=f"b2{tag}")
                nc.sync.dma_start(out=w1_t[:], in_=w1_d[:])
                nc.sync.dma_start(out=b1_t[:], in_=b1_d[:])
                nc.sync.dma_start(out=w2_t[:], in_=w2_d[:])
                nc.sync.dma_start(out=b2_t[:], in_=b2_d[:])
                for s0 in range(0, n_cols, S):
                    s1 = min(s0 + S, n_cols)
                    w = s1 - s0
                    x_t = ap.tile([D, S], BF16, tag="x")
                    nc.sync.dma_start(out=x_t[:, :w], in_=xT_d[:, s0:s1])
                    h_ps = pp.tile([H, S], FP32, tag="h")
                    nc.tensor.matmul(out=h_ps[:, :w], lhsT=w1_t[:],
                                     rhs=x_t[:, :w], start=True, stop=True)
                    h_sb = ap.tile([H, S], BF16, tag="h_sb")
                    nc.scalar.activation(h_sb[:, :w], h_ps[:, :w], AF.Relu,
                                         bias=b1_t[:])
                    m_ps = pp.tile([D, S], FP32, tag="m")
                    nc.tensor.matmul(out=m_ps[:, :w], lhsT=w2_t[:],
                                     rhs=h_sb[:, :w], start=True, stop=True)
                    m_sb = ap.tile([D, S], BF16, tag="m_sb")
                    nc.scalar.activation(m_sb[:, :w], m_ps[:, :w], AF.Identity,
                                         bias=b2_t[:])
                    nc.sync.dma_start(out=out_d[:, s0:s1], in_=m_sb[:, :w])

            mlp(xT, w1, b1, w2, b2, msgT, PC, "v")
            mlp(xcT, w1c, b1c, w2c, b2c, msgcT, N_COLORS, "c")
    nc.compile()
    _cache["A"] = nc
    return nc


# ---------------------------------------------------------------- kernel B
def _build_kernel_b(CE, CEt, Ks):
    key = ("B", CE, CEt, Ks)
    if key in _cache:
        return _cache[key]
    CB = 2 * CE            # chunk columns per full bucket (even + odd)
    CBt = 2 * CEt
    IDXF = NB_FULL * CB * 8 + CBt * 8
    DLC = NB_FULL * CB + CBt

    nc = bacc.Bacc("TRN2", target_bir_lowering=False, debug=False,
                   num_devices=N_CORES, num_swdge_queues=4)
    table = nc.dram_tensor("table", [PAIRS, 2 * D], BF16, kind="ExternalInput")
    msgc = nc.dram_tensor("msgc", [N_COLORS, D], BF16, kind="ExternalInput")
    countT = nc.dram_tensor("countT", [N_COLORS, PC], BF16,
                            kind="ExternalInput")
    xT = nc.dram_tensor("xT", [D, PC], FP32, kind="ExternalInput")
    iota = nc.dram_tensor("iota", [128, WIN * CB], BF16, kind="ExternalInput")
    iota_tl = nc.dram_tensor("iota_tl", [128, WIN * CBt], BF16,
                             kind="ExternalInput")
    dstloc = nc.dram_tensor("dstloc", [128, DLC], BF16, kind="ExternalInput")
    gidx = nc.dram_tensor("gidx", [128, IDXF], I16, kind="ExternalInput")
    outT = nc.dram_tensor("outT", [D, PC], FP32, kind="ExternalOutput")

    wins_full = _windows(CE, BK)
    wins_tail = _windows(CEt, TAIL)

    with tile.TileContext(nc) as tc:
        with (
            tc.tile_pool(name="const", bufs=1) as cp,
            tc.tile_pool(name="gath", bufs=5) as gp,
            tc.tile_pool(name="idx", bufs=NB_FULL + 1) as ip,
            tc.tile_pool(name="oh", bufs=3) as ohp,
            tc.tile_pool(name="ct", bufs=4) as ctp,
            tc.tile_pool(name="ep", bufs=3) as ep,
            tc.tile_pool(name="ps", bufs=4, space="PSUM") as pp,
        ):
            # index tiles first so the first gathers launch immediately
            idx_es, idx_os = [], []
            for b in range(NB_FULL + 1):
                C1 = CE if b < NB_FULL else CEt
                ixoff = b * CB * 8
                idx_e = ip.tile([128, C1 * 8], I16, tag="ide")
                idx_o = ip.tile([128, C1 * 8], I16, tag="ido")
                nc.sync.dma_start(out=idx_e[:],
                                  in_=gidx[:, ixoff:ixoff + C1 * 8])
                nc.sync.dma_start(out=idx_o[:],
                                  in_=gidx[:, ixoff + C1 * 8:ixoff + 2 * C1 * 8])
                idx_es.append(idx_e)
                idx_os.append(idx_o)

            iota_t = cp.tile([128, WIN * CB], BF16, tag="iota")
            nc.scalar.dma_start(out=iota_t[:], in_=iota[:])
            iota_tt = cp.tile([128, WIN * CBt], BF16, tag="iota_tl")
            nc.scalar.dma_start(out=iota_tt[:], in_=iota_tl[:])
            dl_t = cp.tile([128, DLC], BF16, tag="dstloc")
            nc.scalar.dma_start(out=dl_t[:], in_=dstloc[:])
            mc0 = cp.tile([128, D], BF16, tag="msgc0")
            mc1 = cp.tile([128, D], BF16, tag="msgc1")
            nc.scalar.dma_start(out=mc0[:], in_=msgc[0:128, :])
            nc.scalar.dma_start(out=mc1[:], in_=msgc[128:256, :])

            for b in range(NB_FULL + 1):
                full = b < NB_FULL
                W = BK if full else TAIL
                C1 = CE if full else CEt
                wins = wins_full if full else wins_tail
                n0 = b * BK
                dloff = b * CB
                it_t = iota_t if full else iota_tt

                ge = gp.tile([128, C1, D], BF16, tag="ge")
                go = gp.tile([128, C1, D], BF16, tag="go")
                # slots >= K carry idx -1 on every core: the gather ucode
                # trims them (fewer descriptors) and never writes them, so
                # zero that tail region first.
                for g_t, K in ((ge, Ks[b][0]), (go, Ks[b][1])):
                    c0 = K // 128
                    if c0 < C1:
                        # whole chunks: the gather rewrites slots < K after
                        nc.vector.memset(g_t[:, c0:C1, :], 0.0)
                _dma_gather128(nc.gpsimd, ge[:], table[:, 0:D], idx_es[b][:],
                               C1 * 128, Ks[b][0], D, elem_step=2 * D,
                               queue_num=(2 * b) % 4)
                _dma_gather128(nc.gpsimd, go[:], table[:, D:2 * D],
                               idx_os[b][:], C1 * 128, Ks[b][1], D,
                               elem_step=2 * D, queue_num=(2 * b + 1) % 4)

                oh_t = ohp.tile([128, 2 * C1, WIN], BF16, tag="oh")
                nc.vector.tensor_tensor(
                    out=oh_t[:],
                    in0=it_t[:, :2 * C1 * WIN].rearrange(
                        "p (c n) -> p c n", n=WIN),
                    in1=dl_t[:, dloff:dloff + 2 * C1].to_broadcast(
                        [128, 2 * C1, WIN]),
                    op=mybir.AluOpType.is_equal)

                ct0 = ctp.tile([128, W], BF16, tag="ct0")
                ct1 = ctp.tile([128, W], BF16, tag="ct1")
                nc.sync.dma_start(out=ct0[:], in_=countT[0:128, n0:n0 + W])
                nc.sync.dma_start(out=ct1[:], in_=countT[128:256, n0:n0 + W])

                ps = pp.tile([D, W], FP32, tag="agg")
                nc.tensor.matmul(out=ps[:], lhsT=mc0[:], rhs=ct0[:],
                                 start=True, stop=False)
                nc.tensor.matmul(out=ps[:], lhsT=mc1[:], rhs=ct1[:],
                                 start=False, stop=False)
                for c in range(C1):      # even-parity chunks
                    w0 = wins[c]
                    nc.tensor.matmul(out=ps[:, w0:w0 + WIN],
                                     lhsT=ge[:, c, :],
                                     rhs=oh_t[:, c, :],
                                     start=False, stop=False)
                for c in range(C1):      # odd-parity chunks
                    w0 = wins[c]
                    nc.tensor.matmul(out=ps[:, w0:w0 + WIN],
                                     lhsT=go[:, c, :],
                                     rhs=oh_t[:, C1 + c, :],
                                     start=False, stop=(c == C1 - 1))

                xa = ep.tile([D, W], FP32, tag="xa")
                nc.sync.dma_start(out=xa[:], in_=xT[:, n0:n0 + W])
                s_t = ep.tile([D, W], FP32, tag="s")
                nc.vector.tensor_tensor(out=s_t[:], in0=ps[:], in1=xa[:],
                                        op=mybir.AluOpType.add)
                o_t = ep.tile([D, W], FP32, tag="o")
                nc.scalar.activation(o_t[:], s_t[:], AF.Relu)
                nc.sync.dma_start(out=outT[:, n0:n0 + W], in_=o_t[:])
    nc.compile()
    _cache[key] = nc
    return nc


# ------------------------------------------------------------- host logic
def _wrap_idx(flat):
    """dma_gather index layout: [16, n/16] wrapped, replicated to 128 rows."""
    n = flat.shape[0]
    assert n % 16 == 0
    w = flat.reshape(n // 16, 16).T.astype(np.int16)
    return np.tile(w, (8, 1))


def _assign_bucket(d_arr, pidx_arr, width, n_chunks, wins):
    """Greedy window assignment for one (core,bucket,parity) edge group.

    d_arr must be sorted ascending. Returns (slot_idx [C,128] int64,
    slot_dst [C,128] float32) or None if infeasible.
    """
    slot_idx = np.zeros((n_chunks, 128), np.int64)
    slot_dst = np.full((n_chunks, 128), PAD_DST, np.float32)
    if d_arr.shape[0] == 0:
        return slot_idx, slot_dst
    wins_a = np.asarray(wins)
    # lo[d]: first chunk whose window contains d; hi[d]: last such chunk
    ds = np.arange(width)
    lo_map = np.searchsorted(wins_a, ds - (WIN - 1), side="left")
    hi_map = np.searchsorted(wins_a, ds, side="right") - 1
    cnt = np.bincount(d_arr, minlength=width)
    fills = np.zeros(n_chunks, np.int64)
    pos = 0
    for d in range(width):
        need = int(cnt[d])
        if need == 0:
            continue
        c = int(lo_map[d])
        hi = int(hi_map[d])
        while need > 0:
            if c > hi or c >= n_chunks:
                return None
            take = min(need, 128 - int(fills[c]))
            if take > 0:
                f = int(fills[c])
                slot_idx[c, f:f + take] = pidx_arr[pos:pos + take]
                slot_dst[c, f:f + take] = d - wins[c]
                fills[c] += take
                pos += take
                need -= take
            if need > 0:
                c += 1
    return slot_idx, slot_dst


def _used_count(slot_dst):
    """Number of slots up to and including the last real edge (flat order)."""
    used = slot_dst.reshape(-1) != PAD_DST
    nz = np.nonzero(used)[0]
    return int(nz[-1]) + 1 if nz.size else 0


def _prep_vv(src, dst):
    """Bucket/sort/pad vv edges; returns CE, CEt, per-core gidx and dstloc."""
    src = src.astype(np.int64)
    dst = dst.astype(np.int64)
    core = dst // PC
    d_in_core = dst - core * PC
    bucket = np.minimum(d_in_core // BK, NB_FULL)
    d_local = d_in_core - bucket * BK
    parity = src & 1
    pidx = src >> 1

    key = ((core * (NB_FULL + 1) + bucket) * 2 + parity)
    order = np.lexsort((d_local, key))
    key_s = key[order]
    d_s = d_local[order]
    p_s = pidx[order]
    n_groups = N_CORES * (NB_FULL + 1) * 2
    counts = np.bincount(key_s, minlength=n_groups)
    starts = np.concatenate([[0], np.cumsum(counts)[:-1]])

    # global chunk counts
    cnt_full = counts.reshape(N_CORES, NB_FULL + 1, 2)
    CE = max(1, int(np.ceil(cnt_full[:, :NB_FULL, :].max() / 128)))
    CEt = max(1, int(np.ceil(cnt_full[:, NB_FULL, :].max() / 128)))

    for _ in range(4):
        wins_full = _windows(CE, BK)
        wins_tail = _windows(CEt, TAIL)
        res = [[None] * (2 * (NB_FULL + 1)) for _ in range(N_CORES)]
        ok = True
        for k in range(N_CORES):
            for b in range(NB_FULL + 1):
                fullb = b < NB_FULL
                width = BK if fullb else TAIL
                C1 = CE if fullb else CEt
                wins = wins_full if fullb else wins_tail
                for par in range(2):
                    gk = (k * (NB_FULL + 1) + b) * 2 + par
                    s0, c0 = starts[gk], counts[gk]
                    r = _assign_bucket(d_s[s0:s0 + c0], p_s[s0:s0 + c0],
                                       width, C1, wins)
                    if r is None:
                        ok = False
                        break
                    res[k][b * 2 + par] = r
                if not ok:
                    break
            if not ok:
                break
        if ok:
            break
        CE += 1
        CEt += 1
    else:
        raise RuntimeError("window assignment infeasible")

    # core-uniform trim counts: the gather ucode trims trailing -1 indices
    # and the decode reserves ring space from num_idxs_reg, so the trimmed
    # count must be identical on every core.
    Ks = []
    for b in range(NB_FULL + 1):
        kpair = []
        for par in range(2):
            n = max(_used_count(res[k][b * 2 + par][1])
                    for k in range(N_CORES))
            if not _TRIM_TAIL:
                n = (CE if b < NB_FULL else CEt) * 128
            kpair.append(max(n, 128))
        Ks.append(tuple(kpair))
    Ks = tuple(Ks)

    gidx, dstloc = [], []
    for k in range(N_CORES):
        parts_i, parts_d = [], []
        for b in range(NB_FULL + 1):
            ie, de = res[k][b * 2 + 0]
            io, do = res[k][b * 2 + 1]
            fe = ie.reshape(-1).copy()
            fo = io.reshape(-1).copy()
            fe[Ks[b][0]:] = -1
            fo[Ks[b][1]:] = -1
            parts_i.append(np.concatenate(
                [_wrap_idx(fe), _wrap_idx(fo)], axis=1))
            dl = np.concatenate([de, do], axis=0).T    # [128, 2*C1]
            parts_d.append(np.ascontiguousarray(dl))
        gidx.append(np.concatenate(parts_i, axis=1))
        dstloc.append(np.concatenate(parts_d, axis=1).astype(NPBF16))
    return CE, CEt, gidx, dstloc, Ks


def kernel(x_v, x_c, W1v, b1v, W2v, b2v, W1c, b1c, W2c, b2c,
           src_vv, dst_vv, src_vc, dst_vc):
    x_v = np.asarray(x_v, np.float32)
    x_c = np.asarray(x_c, np.float32)
    src_vv = np.asarray(src_vv, np.int32)
    dst_vv = np.asarray(dst_vv, np.int32)
    src_vc = np.asarray(src_vc, np.int32)
    dst_vc = np.asarray(dst_vc, np.int32)

    # ---------------- kernel A: message tables ----------------
    xT_full = np.zeros((D, NP), np.float32)
    xT_full[:, :N_NODES] = x_v.T
    a_common = {
        "w1": np.asarray(W1v, np.float32).astype(NPBF16),
        "b1": np.asarray(b1v, np.float32).reshape(H, 1),
        "w2": np.asarray(W2v, np.float32).astype(NPBF16),
        "b2": np.asarray(b2v, np.float32).reshape(D, 1),
        "xcT": np.ascontiguousarray(x_c.T).astype(NPBF16),
        "w1c": np.asarray(W1c, np.float32).astype(NPBF16),
        "b1c": np.asarray(b1c, np.float32).reshape(H, 1),
        "w2c": np.asarray(W2c, np.float32).astype(NPBF16),
        "b2c": np.asarray(b2c, np.float32).reshape(D, 1),
    }
    in_maps_a = []
    for k in range(N_CORES):
        m = dict(a_common)
        m["xT"] = np.ascontiguousarray(
            xT_full[:, k * PC:(k + 1) * PC]).astype(NPBF16)
        in_maps_a.append(m)
    nc_a = _build_kernel_a()
    res_a = _run(nc_a, in_maps_a, "A")

    msg = np.concatenate(
        [np.asarray(res_a[k]["msgT"]) for k in range(N_CORES)], axis=1).T
    msg_c = np.ascontiguousarray(np.asarray(res_a[0]["msgcT"]).T)  # [256,64]

    table = np.zeros((PAIRS, 2 * D), NPBF16)
    table[:NP // 2] = msg.reshape(NP // 2, 2 * D)

    # ---------------- host: index prep ----------------
    CE, CEt, gidx, dstloc, Ks = _prep_vv(src_vv, dst_vv)

    cnt = np.bincount(src_vc.astype(np.int64) * NP + dst_vc,
                      minlength=N_COLORS * NP).reshape(N_COLORS, NP)
    countT = cnt.astype(NPBF16)

    CB = 2 * CE
    CBt = 2 * CEt
    iota = np.tile(np.arange(WIN, dtype=np.float32),
                   (128, CB)).astype(NPBF16)
    iota_tl = np.tile(np.arange(WIN, dtype=np.float32),
                      (128, CBt)).astype(NPBF16)

    # ---------------- kernel B: gather + scatter + epilogue ----------------
    in_maps_b = []
    for k in range(N_CORES):
        in_maps_b.append({
            "table": table,
            "msgc": np.ascontiguousarray(msg_c.astype(NPBF16)),
            "countT": np.ascontiguousarray(countT[:, k * PC:(k + 1) * PC]),
            "xT": np.ascontiguousarray(xT_full[:, k * PC:(k + 1) * PC]),
            "iota": iota,
            "iota_tl": iota_tl,
            "dstloc": dstloc[k],
            "gidx": gidx[k],
        })
    nc_b = _build_kernel_b(CE, CEt, Ks)
    res_b = _run(nc_b, in_maps_b, "B")

    outT = np.concatenate(
        [np.asarray(res_b[k]["outT"]) for k in range(N_CORES)], axis=1)
    return np.ascontiguousarray(outT.T[:N_NODES]).astype(np.float32)


# revision 34
# speedup vs baseline: 1.0684x; 1.0554x over previous
"""GNN message-passing block on 8 Trainium2 NeuronCores.

Math: out[n] = relu(x_v[n] + agg_v[n] + agg_c[n])
    agg_v = segment_sum(MLPv(x_v)[src_vv], dst_vv)   (messages depend on src only)
    agg_c = Count @ MLPc(x_c)          (256 colors -> dense count matmul)

Design (v2, bf16):
  * Kernel A (node-sharded): computes the 50k-row message table in bf16.
  * Kernel B (dst-sharded): per-edge gather of bf16 pair-rows (256 B each,
    the dma_gather minimum element) + scatter-add via one-hot matmuls.
  * Edges are bucketed by 512-node dst range (one PSUM bank per bucket),
    split by src parity (a chunk's matmul reads the correct 64-column half
    of the gathered pair), and dst-sorted so each 128-edge chunk only
    covers a narrow 64-node window at a COMPILE-TIME offset (the window
    ladder w_c = 16c - 5.6*sqrt(c) is feasible w.h.p. for uniform edges;
    the host greedy verifies and bumps the chunk count on failure).
    The two full-width color-count matmuls run first with start=True so
    every PSUM element is initialized regardless of window coverage gaps.
  * The 13 per-bucket gathers rotate across all 4 SWDGE queues with deep
    buffering so descriptor generation and SDMA drain never go idle.
"""

import math

import numpy as np

import concourse.bacc as bacc
import concourse.mybir as mybir
import concourse.tile as tile
from concourse import ap_utils
from concourse._compat import exact_div
from concourse.bass import MemorySpace
from concourse.bass_utils import run_bass_kernel_spmd

FP32 = mybir.dt.float32
BF16 = mybir.dt.bfloat16
I16 = mybir.dt.int16
AF = mybir.ActivationFunctionType
NPBF16 = mybir.dt.np(BF16)

N_CORES = 8
N_NODES = 50000
N_COLORS = 256
D = 64
H = 128
NP = 50176              # nodes padded to 392 tiles of 128
PC = NP // N_CORES      # 6272 nodes per core
BK = 512                # bucket = one PSUM bank of fp32
NB_FULL = PC // BK      # 12 full buckets; tail bucket of 128 nodes
TAIL = PC - NB_FULL * BK
PAIRS = NP // 2 + 128   # bf16 pair-row table rows (padded)
WIN = 64                # one-hot window width
PAD_DST = 100.0

PROFILE = False
LAST_EXEC_NS = {}
_TRIM_TAIL = True
GP_BUFS = 5           # gather pool depth; buckets < GP_BUFS are untrimmed

_cache = {}


def _run(nc, in_maps, label):
    kwargs = {}
    if PROFILE:
        kwargs = dict(trace=True, trace_cores=[0])
    try:
        res = run_bass_kernel_spmd(nc, in_maps, list(range(N_CORES)), **kwargs)
    except Exception:
        if not kwargs:
            raise
        res = run_bass_kernel_spmd(nc, in_maps, list(range(N_CORES)))
    LAST_EXEC_NS[label] = res.exec_time_ns
    return res.results


def _dma_gather128(eng, out_ap, in_ap, idxs_ap, num_idxs, num_idxs_reg,
                   elem_size, elem_step, queue_num):
    """bass dma_gather for 128-byte elements.

    Identical to bass.GpSimd.dma_gather (non-transpose, DRAM source,
    immediate trigger) except the element only has to be a multiple of
    128 B; the row stride must still be a multiple of 256 B, which is the
    only granularity the descriptor ucode actually requires
    (stride_bytes_256).  The ucode's non-transpose path emits one plain
    CME descriptor of elem_size bytes per index, so 128 B is fine.
    """
    eng._assert_queue_num(queue_num)
    assert idxs_ap.dtype == mybir.dt.int16
    assert in_ap.dtype == out_ap.dtype
    assert in_ap.space == MemorySpace.DRAM
    assert idxs_ap.space == MemorySpace.SBUF
    assert out_ap.space == MemorySpace.SBUF
    elem_size_bytes = elem_size * mybir.dt.size(in_ap.dtype)
    assert elem_size_bytes % 128 == 0
    assert ap_utils.ap_is_contiguous(in_ap.ap[1:])
    assert ap_utils.ap_is_contiguous(out_ap.ap[1:])
    assert ap_utils.ap_is_contiguous(idxs_ap.ap[1:])
    assert in_ap.ap[-1][1] == out_ap.ap[-1][1] == elem_size
    assert out_ap.ap[0][1] * out_ap.ap[1][1] == num_idxs
    assert in_ap.ap[0][0] == elem_step
    stride_bytes_256 = exact_div(elem_step * mybir.dt.size(in_ap.dtype), 256)
    assert stride_bytes_256 < 256
    return eng.add_instruction(
        mybir.InstDMAGatherAnt(
            name=eng.bass.get_next_instruction_name(),
            ins=[
                *eng.lower_ap_dma(in_ap, for_custom_bir_dma=True),
                eng.lower_ap(idxs_ap),
                eng.lower_val_access(eng.to_reg(num_idxs_reg)),
            ],
            outs=[eng.lower_ap(out_ap)],
            transpose=False,
            num_idxs=num_idxs,
            elem_size=elem_size,
            stride_bytes_256=stride_bytes_256,
            gen_mode=0,
            single_packet=False,
            queue_num=queue_num,
            sbuf_tokens_per_rank=0,
            sbuf_free_dim_per_rank=0,
            sbuf_free_dim_pad_per_rank=0,
            sbuf_byte_offset=0,
        )
    )


def _windows(n_chunks, width):
    """Compile-time window offsets; clamped ascending ladder."""
    top = width - WIN
    ws = []
    for c in range(n_chunks):
        w = int(round(16 * c - 5.6 * math.sqrt(c)))
        ws.append(min(top, max(0, w)))
    return ws


# ---------------------------------------------------------------- kernel A
def _build_kernel_a():
    if "A" in _cache:
        return _cache["A"]
    nc = bacc.Bacc("TRN2", target_bir_lowering=False, debug=False,
                   num_devices=N_CORES)
    xT = nc.dram_tensor("xT", [D, PC], BF16, kind="ExternalInput")
    w1 = nc.dram_tensor("w1", [D, H], BF16, kind="ExternalInput")
    b1 = nc.dram_tensor("b1", [H, 1], FP32, kind="ExternalInput")
    w2 = nc.dram_tensor("w2", [H, D], BF16, kind="ExternalInput")
    b2 = nc.dram_tensor("b2", [D, 1], FP32, kind="ExternalInput")
    xcT = nc.dram_tensor("xcT", [D, N_COLORS], BF16, kind="ExternalInput")
    w1c = nc.dram_tensor("w1c", [D, H], BF16, kind="ExternalInput")
    b1c = nc.dram_tensor("b1c", [H, 1], FP32, kind="ExternalInput")
    w2c = nc.dram_tensor("w2c", [H, D], BF16, kind="ExternalInput")
    b2c = nc.dram_tensor("b2c", [D, 1], FP32, kind="ExternalInput")
    msgT = nc.dram_tensor("msgT", [D, PC], BF16, kind="ExternalOutput")
    msgcT = nc.dram_tensor("msgcT", [D, N_COLORS], BF16, kind="ExternalOutput")

    S = 512
    with tile.TileContext(nc) as tc:
        with (
            tc.tile_pool(name="w", bufs=1) as wp,
            tc.tile_pool(name="act", bufs=4) as ap,
            tc.tile_pool(name="ps", bufs=4, space="PSUM") as pp,
        ):
            def mlp(xT_d, w1_d, b1_d, w2_d, b2_d, out_d, n_cols, tag):
                w1_t = wp.tile([D, H], BF16, tag=f"w1{tag}")
                b1_t = wp.tile([H, 1], FP32, tag=f"b1{tag}")
                w2_t = wp.tile([H, D], BF16, tag=f"w2{tag}")
                b2_t = wp.tile([D, 1], FP32, tag=f"b2{tag}")
                nc.sync.dma_start(out=w1_t[:], in_=w1_d[:])
                nc.sync.dma_start(out=b1_t[:], in_=b1_d[:])
                nc.sync.dma_start(out=w2_t[:], in_=w2_d[:])
                nc.sync.dma_start(out=b2_t[:], in_=b2_d[:])
                for s0 in range(0, n_cols, S):
                    s1 = min(s0 + S, n_cols)
                    w = s1 - s0
                    x_t = ap.tile([D, S], BF16, tag="x")
                    nc.sync.dma_start(out=x_t[:, :w], in_=xT_d[:, s0:s1])
                    h_ps = pp.tile([H, S], FP32, tag="h")
                    nc.tensor.matmul(out=h_ps[:, :w], lhsT=w1_t[:],
                                     rhs=x_t[:, :w], start=True, stop=True)
                    h_sb = ap.tile([H, S], BF16, tag="h_sb")
                    nc.scalar.activation(h_sb[:, :w], h_ps[:, :w], AF.Relu,
                                         bias=b1_t[:])
                    m_ps = pp.tile([D, S], FP32, tag="m")
                    nc.tensor.matmul(out=m_ps[:, :w], lhsT=w2_t[:],
                                     rhs=h_sb[:, :w], start=True, stop=True)
                    m_sb = ap.tile([D, S], BF16, tag="m_sb")
                    nc.scalar.activation(m_sb[:, :w], m_ps[:, :w], AF.Identity,
                                         bias=b2_t[:])
                    nc.sync.dma_start(out=out_d[:, s0:s1], in_=m_sb[:, :w])

            mlp(xT, w1, b1, w2, b2, msgT, PC, "v")
            mlp(xcT, w1c, b1c, w2c, b2c, msgcT, N_COLORS, "c")
    nc.compile()
    _cache["A"] = nc
    return nc


# ---------------------------------------------------------------- kernel B
def _build_kernel_b(CE, CEt, Ks):
    key = ("B", CE, CEt, Ks)
    if key in _cache:
        return _cache[key]
    CB = 2 * CE            # chunk columns per full bucket (even + odd)
    CBt = 2 * CEt
    IDXF = NB_FULL * CB * 8 + CBt * 8
    DLC = NB_FULL * CB + CBt

    nc = bacc.Bacc("TRN2", target_bir_lowering=False, debug=False,
                   num_devices=N_CORES, num_swdge_queues=4)
    table = nc.dram_tensor("table", [PAIRS, 2 * D], BF16, kind="ExternalInput")
    msgc = nc.dram_tensor("msgc", [N_COLORS, D], BF16, kind="ExternalInput")
    countT = nc.dram_tensor("countT", [N_COLORS, PC], BF16,
                            kind="ExternalInput")
    xT = nc.dram_tensor("xT", [D, PC], FP32, kind="ExternalInput")
    iota = nc.dram_tensor("iota", [128, WIN * CB], BF16, kind="ExternalInput")
    iota_tl = nc.dram_tensor("iota_tl", [128, WIN * CBt], BF16,
                             kind="ExternalInput")
    dstloc = nc.dram_tensor("dstloc", [128, DLC], BF16, kind="ExternalInput")
    gidx = nc.dram_tensor("gidx", [128, IDXF], I16, kind="ExternalInput")
    outT = nc.dram_tensor("outT", [D, PC], FP32, kind="ExternalOutput")

    wins_full = _windows(CE, BK)
    wins_tail = _windows(CEt, TAIL)

    with tile.TileContext(nc) as tc:
        with (
            tc.tile_pool(name="const", bufs=1) as cp,
            tc.tile_pool(name="gath", bufs=GP_BUFS) as gp,
            tc.tile_pool(name="idx", bufs=NB_FULL + 1) as ip,
            tc.tile_pool(name="oh", bufs=3) as ohp,
            tc.tile_pool(name="ct", bufs=4) as ctp,
            tc.tile_pool(name="ep", bufs=3) as ep,
            tc.tile_pool(name="ps", bufs=4, space="PSUM") as pp,
        ):
            # index tiles first so the first gathers launch immediately
            idx_es, idx_os = [], []
            for b in range(NB_FULL + 1):
                C1 = CE if b < NB_FULL else CEt
                ixoff = b * CB * 8
                idx_e = ip.tile([128, C1 * 8], I16, tag="ide")
                idx_o = ip.tile([128, C1 * 8], I16, tag="ido")
                nc.sync.dma_start(out=idx_e[:],
                                  in_=gidx[:, ixoff:ixoff + C1 * 8])
                nc.sync.dma_start(out=idx_o[:],
                                  in_=gidx[:, ixoff + C1 * 8:ixoff + 2 * C1 * 8])
                idx_es.append(idx_e)
                idx_os.append(idx_o)

            iota_t = cp.tile([128, WIN * CB], BF16, tag="iota")
            nc.scalar.dma_start(out=iota_t[:], in_=iota[:])
            iota_tt = cp.tile([128, WIN * CBt], BF16, tag="iota_tl")
            nc.scalar.dma_start(out=iota_tt[:], in_=iota_tl[:])
            dl_t = cp.tile([128, DLC], BF16, tag="dstloc")
            nc.scalar.dma_start(out=dl_t[:], in_=dstloc[:])
            mc0 = cp.tile([128, D], BF16, tag="msgc0")
            mc1 = cp.tile([128, D], BF16, tag="msgc1")
            nc.scalar.dma_start(out=mc0[:], in_=msgc[0:128, :])
            nc.scalar.dma_start(out=mc1[:], in_=msgc[128:256, :])

            for b in range(NB_FULL + 1):
                full = b < NB_FULL
                W = BK if full else TAIL
                C1 = CE if full else CEt
                wins = wins_full if full else wins_tail
                n0 = b * BK
                dloff = b * CB
                it_t = iota_t if full else iota_tt

                ge = gp.tile([128, C1, D], BF16, tag="ge")
                go = gp.tile([128, C1, D], BF16, tag="go")
                # Trailing slots >= K carry idx -1 on every core and are
                # skipped by the gather ucode.  Only trimmed for buckets
                # whose recycled pool buffer already holds finite values
                # (their zero one-hot columns then contribute exactly 0);
                # the first GP_BUFS buckets gather every slot.
                _dma_gather128(nc.gpsimd, ge[:], table[:, 0:D], idx_es[b][:],
                               C1 * 128, Ks[b][0], D, elem_step=2 * D,
                               queue_num=(2 * b) % 4)
                _dma_gather128(nc.gpsimd, go[:], table[:, D:2 * D],
                               idx_os[b][:], C1 * 128, Ks[b][1], D,
                               elem_step=2 * D, queue_num=(2 * b + 1) % 4)

                oh_t = ohp.tile([128, 2 * C1, WIN], BF16, tag="oh")
                nc.vector.tensor_tensor(
                    out=oh_t[:],
                    in0=it_t[:, :2 * C1 * WIN].rearrange(
                        "p (c n) -> p c n", n=WIN),
                    in1=dl_t[:, dloff:dloff + 2 * C1].to_broadcast(
                        [128, 2 * C1, WIN]),
                    op=mybir.AluOpType.is_equal)

                ct0 = ctp.tile([128, W], BF16, tag="ct0")
                ct1 = ctp.tile([128, W], BF16, tag="ct1")
                nc.sync.dma_start(out=ct0[:], in_=countT[0:128, n0:n0 + W])
                nc.sync.dma_start(out=ct1[:], in_=countT[128:256, n0:n0 + W])

                ps = pp.tile([D, W], FP32, tag="agg")
                nc.tensor.matmul(out=ps[:], lhsT=mc0[:], rhs=ct0[:],
                                 start=True, stop=False)
                nc.tensor.matmul(out=ps[:], lhsT=mc1[:], rhs=ct1[:],
                                 start=False, stop=False)
                for c in range(C1):      # even-parity chunks
                    w0 = wins[c]
                    nc.tensor.matmul(out=ps[:, w0:w0 + WIN],
                                     lhsT=ge[:, c, :],
                                     rhs=oh_t[:, c, :],
                                     start=False, stop=False)
                for c in range(C1):      # odd-parity chunks
                    w0 = wins[c]
                    nc.tensor.matmul(out=ps[:, w0:w0 + WIN],
                                     lhsT=go[:, c, :],
                                     rhs=oh_t[:, C1 + c, :],
                                     start=False, stop=(c == C1 - 1))

                xa = ep.tile([D, W], FP32, tag="xa")
                nc.sync.dma_start(out=xa[:], in_=xT[:, n0:n0 + W])
                s_t = ep.tile([D, W], FP32, tag="s")
                nc.vector.tensor_tensor(out=s_t[:], in0=ps[:], in1=xa[:],
                                        op=mybir.AluOpType.add)
                o_t = ep.tile([D, W], FP32, tag="o")
                nc.scalar.activation(o_t[:], s_t[:], AF.Relu)
                nc.sync.dma_start(out=outT[:, n0:n0 + W], in_=o_t[:])
    nc.compile()
    _cache[key] = nc
    return nc


# ------------------------------------------------------------- host logic
def _wrap_idx(flat):
    """dma_gather index layout: [16, n/16] wrapped, replicated to 128 rows."""
    n = flat.shape[0]
    assert n % 16 == 0
    w = flat.reshape(n // 16, 16).T.astype(np.int16)
    return np.tile(w, (8, 1))


def _assign_bucket(d_arr, pidx_arr, width, n_chunks, wins):
    """Greedy window assignment for one (core,bucket,parity) edge group.

    d_arr must be sorted ascending. Returns (slot_idx [C,128] int64,
    slot_dst [C,128] float32) or None if infeasible.
    """
    slot_idx = np.zeros((n_chunks, 128), np.int64)
    slot_dst = np.full((n_chunks, 128), PAD_DST, np.float32)
    if d_arr.shape[0] == 0:
        return slot_idx, slot_dst
    wins_a = np.asarray(wins)
    # lo[d]: first chunk whose window contains d; hi[d]: last such chunk
    ds = np.arange(width)
    lo_map = np.searchsorted(wins_a, ds - (WIN - 1), side="left")
    hi_map = np.searchsorted(wins_a, ds, side="right") - 1
    cnt = np.bincount(d_arr, minlength=width)
    fills = np.zeros(n_chunks, np.int64)
    pos = 0
    for d in range(width):
        need = int(cnt[d])
        if need == 0:
            continue
        c = int(lo_map[d])
        hi = int(hi_map[d])
        while need > 0:
            if c > hi or c >= n_chunks:
                return None
            take = min(need, 128 - int(fills[c]))
            if take > 0:
                f = int(fills[c])
                slot_idx[c, f:f + take] = pidx_arr[pos:pos + take]
                slot_dst[c, f:f + take] = d - wins[c]
                fills[c] += take
                pos += take
                need -= take
            if need > 0:
                c += 1
    return slot_idx, slot_dst


def _used_count(slot_dst):
    """Number of slots up to and including the last real edge (flat order)."""
    used = slot_dst.reshape(-1) != PAD_DST
    nz = np.nonzero(used)[0]
    return int(nz[-1]) + 1 if nz.size else 0


def _prep_vv(src, dst):
    """Bucket/sort/pad vv edges; returns CE, CEt, per-core gidx and dstloc."""
    src = src.astype(np.int64)
    dst = dst.astype(np.int64)
    core = dst // PC
    d_in_core = dst - core * PC
    bucket = np.minimum(d_in_core // BK, NB_FULL)
    d_local = d_in_core - bucket * BK
    parity = src & 1
    pidx = src >> 1

    key = ((core * (NB_FULL + 1) + bucket) * 2 + parity)
    order = np.lexsort((d_local, key))
    key_s = key[order]
    d_s = d_local[order]
    p_s = pidx[order]
    n_groups = N_CORES * (NB_FULL + 1) * 2
    counts = np.bincount(key_s, minlength=n_groups)
    starts = np.concatenate([[0], np.cumsum(counts)[:-1]])

    # global chunk counts
    cnt_full = counts.reshape(N_CORES, NB_FULL + 1, 2)
    CE = max(1, int(np.ceil(cnt_full[:, :NB_FULL, :].max() / 128)))
    CEt = max(1, int(np.ceil(cnt_full[:, NB_FULL, :].max() / 128)))

    for _ in range(4):
        wins_full = _windows(CE, BK)
        wins_tail = _windows(CEt, TAIL)
        res = [[None] * (2 * (NB_FULL + 1)) for _ in range(N_CORES)]
        ok = True
        for k in range(N_CORES):
            for b in range(NB_FULL + 1):
                fullb = b < NB_FULL
                width = BK if fullb else TAIL
                C1 = CE if fullb else CEt
                wins = wins_full if fullb else wins_tail
                for par in range(2):
                    gk = (k * (NB_FULL + 1) + b) * 2 + par
                    s0, c0 = starts[gk], counts[gk]
                    r = _assign_bucket(d_s[s0:s0 + c0], p_s[s0:s0 + c0],
                                       width, C1, wins)
                    if r is None:
                        ok = False
                        break
                    res[k][b * 2 + par] = r
                if not ok:
                    break
            if not ok:
                break
        if ok:
            break
        CE += 1
        CEt += 1
    else:
        raise RuntimeError("window assignment infeasible")

    # core-uniform trim counts: the gather ucode trims trailing -1 indices
    # and the decode reserves ring space from num_idxs_reg, so the trimmed
    # count must be identical on every core.
    Ks = []
    for b in range(NB_FULL + 1):
        kpair = []
        for par in range(2):
            n = max(_used_count(res[k][b * 2 + par][1])
                    for k in range(N_CORES))
            if not _TRIM_TAIL or b < GP_BUFS:
                n = (CE if b < NB_FULL else CEt) * 128
            kpair.append(max(n, 128))
        Ks.append(tuple(kpair))
    Ks = tuple(Ks)

    gidx, dstloc = [], []
    for k in range(N_CORES):
        parts_i, parts_d = [], []
        for b in range(NB_FULL + 1):
            ie, de = res[k][b * 2 + 0]
            io, do = res[k][b * 2 + 1]
            fe = ie.reshape(-1).copy()
            fo = io.reshape(-1).copy()
            fe[Ks[b][0]:] = -1
            fo[Ks[b][1]:] = -1
            parts_i.append(np.concatenate(
                [_wrap_idx(fe), _wrap_idx(fo)], axis=1))
            dl = np.concatenate([de, do], axis=0).T    # [128, 2*C1]
            parts_d.append(np.ascontiguousarray(dl))
        gidx.append(np.concatenate(parts_i, axis=1))
        dstloc.append(np.concatenate(parts_d, axis=1).astype(NPBF16))
    return CE, CEt, gidx, dstloc, Ks


def kernel(x_v, x_c, W1v, b1v, W2v, b2v, W1c, b1c, W2c, b2c,
           src_vv, dst_vv, src_vc, dst_vc):
    x_v = np.asarray(x_v, np.float32)
    x_c = np.asarray(x_c, np.float32)
    src_vv = np.asarray(src_vv, np.int32)
    dst_vv = np.asarray(dst_vv, np.int32)
    src_vc = np.asarray(src_vc, np.int32)
    dst_vc = np.asarray(dst_vc, np.int32)

    # ---------------- kernel A: message tables ----------------
    xT_full = np.zeros((D, NP), np.float32)
    xT_full[:, :N_NODES] = x_v.T
    a_common = {
        "w1": np.asarray(W1v, np.float32).astype(NPBF16),
        "b1": np.asarray(b1v, np.float32).reshape(H, 1),
        "w2": np.asarray(W2v, np.float32).astype(NPBF16),
        "b2": np.asarray(b2v, np.float32).reshape(D, 1),
        "xcT": np.ascontiguousarray(x_c.T).astype(NPBF16),
        "w1c": np.asarray(W1c, np.float32).astype(NPBF16),
        "b1c": np.asarray(b1c, np.float32).reshape(H, 1),
        "w2c": np.asarray(W2c, np.float32).astype(NPBF16),
        "b2c": np.asarray(b2c, np.float32).reshape(D, 1),
    }
    in_maps_a = []
    for k in range(N_CORES):
        m = dict(a_common)
        m["xT"] = np.ascontiguousarray(
            xT_full[:, k * PC:(k + 1) * PC]).astype(NPBF16)
        in_maps_a.append(m)
    nc_a = _build_kernel_a()
    res_a = _run(nc_a, in_maps_a, "A")

    msg = np.concatenate(
        [np.asarray(res_a[k]["msgT"]) for k in range(N_CORES)], axis=1).T
    msg_c = np.ascontiguousarray(np.asarray(res_a[0]["msgcT"]).T)  # [256,64]

    table = np.zeros((PAIRS, 2 * D), NPBF16)
    table[:NP // 2] = msg.reshape(NP // 2, 2 * D)

    # ---------------- host: index prep ----------------
    CE, CEt, gidx, dstloc, Ks = _prep_vv(src_vv, dst_vv)

    cnt = np.bincount(src_vc.astype(np.int64) * NP + dst_vc,
                      minlength=N_COLORS * NP).reshape(N_COLORS, NP)
    countT = cnt.astype(NPBF16)

    CB = 2 * CE
    CBt = 2 * CEt
    iota = np.tile(np.arange(WIN, dtype=np.float32),
                   (128, CB)).astype(NPBF16)
    iota_tl = np.tile(np.arange(WIN, dtype=np.float32),
                      (128, CBt)).astype(NPBF16)

    # ---------------- kernel B: gather + scatter + epilogue ----------------
    in_maps_b = []
    for k in range(N_CORES):
        in_maps_b.append({
            "table": table,
            "msgc": np.ascontiguousarray(msg_c.astype(NPBF16)),
            "countT": np.ascontiguousarray(countT[:, k * PC:(k + 1) * PC]),
            "xT": np.ascontiguousarray(xT_full[:, k * PC:(k + 1) * PC]),
            "iota": iota,
            "iota_tl": iota_tl,
            "dstloc": dstloc[k],
            "gidx": gidx[k],
        })
    nc_b = _build_kernel_b(CE, CEt, Ks)
    res_b = _run(nc_b, in_maps_b, "B")

    outT = np.concatenate(
        [np.asarray(res_b[k]["outT"]) for k in range(N_CORES)], axis=1)
    return np.ascontiguousarray(outT.T[:N_NODES]).astype(np.float32)


# revision 37
# speedup vs baseline: 1.2561x; 1.1756x over previous
"""GNN message-passing block on 8 Trainium2 NeuronCores.

Math: out[n] = relu(x_v[n] + agg_v[n] + agg_c[n])
    agg_v = segment_sum(MLPv(x_v)[src_vv], dst_vv)   (messages depend on src only)
    agg_c = Count @ MLPc(x_c)          (256 colors -> dense count matmul)

Design (v2, bf16):
  * Kernel A (node-sharded): computes the 50k-row message table in bf16.
  * Kernel B (dst-sharded): per-edge gather of bf16 pair-rows (256 B each,
    the dma_gather minimum element) + scatter-add via one-hot matmuls.
  * Edges are bucketed by 512-node dst range (one PSUM bank per bucket),
    split by src parity (a chunk's matmul reads the correct 64-column half
    of the gathered pair), and dst-sorted so each 128-edge chunk only
    covers a narrow 64-node window at a COMPILE-TIME offset (the window
    ladder w_c = 16c - 5.6*sqrt(c) is feasible w.h.p. for uniform edges;
    the host greedy verifies and bumps the chunk count on failure).
    The two full-width color-count matmuls run first with start=True so
    every PSUM element is initialized regardless of window coverage gaps.
  * The 13 per-bucket gathers rotate across all 4 SWDGE queues with deep
    buffering so descriptor generation and SDMA drain never go idle.
"""

import math

import numpy as np

import concourse.bacc as bacc
import concourse.mybir as mybir
import concourse.tile as tile
from concourse import ap_utils
from concourse._compat import exact_div
from concourse.bass import MemorySpace
from concourse.bass_utils import run_bass_kernel_spmd

FP32 = mybir.dt.float32
BF16 = mybir.dt.bfloat16
I16 = mybir.dt.int16
AF = mybir.ActivationFunctionType
NPBF16 = mybir.dt.np(BF16)

N_CORES = 8
N_NODES = 50000
N_COLORS = 256
D = 64
H = 128
NP = 50176              # nodes padded to 392 tiles of 128
PC = NP // N_CORES      # 6272 nodes per core
BK = 512                # bucket = one PSUM bank of fp32
NB_FULL = PC // BK      # 12 full buckets; tail bucket of 128 nodes
TAIL = PC - NB_FULL * BK
PAIRS = NP // 2 + 128   # bf16 pair-row table rows (padded)
WIN = 64                # one-hot window width
PAD_DST = 100.0

PROFILE = False
LAST_EXEC_NS = {}
_TRIM_TAIL = False
GP_BUFS = 5           # gather pool depth; buckets < GP_BUFS are untrimmed

_cache = {}


def _run(nc, in_maps, label):
    kwargs = {}
    if PROFILE:
        kwargs = dict(trace=True, trace_cores=[0])
    try:
        res = run_bass_kernel_spmd(nc, in_maps, list(range(N_CORES)), **kwargs)
    except Exception:
        if not kwargs:
            raise
        res = run_bass_kernel_spmd(nc, in_maps, list(range(N_CORES)))
    LAST_EXEC_NS[label] = res.exec_time_ns
    return res.results


def _dma_gather128(eng, out_ap, in_ap, idxs_ap, num_idxs, num_idxs_reg,
                   elem_size, elem_step, queue_num):
    """bass dma_gather for 128-byte elements.

    Identical to bass.GpSimd.dma_gather (non-transpose, DRAM source,
    immediate trigger) except the element only has to be a multiple of
    128 B; the row stride must still be a multiple of 256 B, which is the
    only granularity the descriptor ucode actually requires
    (stride_bytes_256).  The ucode's non-transpose path emits one plain
    CME descriptor of elem_size bytes per index, so 128 B is fine.
    """
    eng._assert_queue_num(queue_num)
    assert idxs_ap.dtype == mybir.dt.int16
    assert in_ap.dtype == out_ap.dtype
    assert in_ap.space == MemorySpace.DRAM
    assert idxs_ap.space == MemorySpace.SBUF
    assert out_ap.space == MemorySpace.SBUF
    elem_size_bytes = elem_size * mybir.dt.size(in_ap.dtype)
    assert elem_size_bytes % 128 == 0
    assert ap_utils.ap_is_contiguous(in_ap.ap[1:])
    assert ap_utils.ap_is_contiguous(out_ap.ap[1:])
    assert ap_utils.ap_is_contiguous(idxs_ap.ap[1:])
    assert in_ap.ap[-1][1] == out_ap.ap[-1][1] == elem_size
    assert out_ap.ap[0][1] * out_ap.ap[1][1] == num_idxs
    assert in_ap.ap[0][0] == elem_step
    stride_bytes_256 = exact_div(elem_step * mybir.dt.size(in_ap.dtype), 256)
    assert stride_bytes_256 < 256
    return eng.add_instruction(
        mybir.InstDMAGatherAnt(
            name=eng.bass.get_next_instruction_name(),
            ins=[
                *eng.lower_ap_dma(in_ap, for_custom_bir_dma=True),
                eng.lower_ap(idxs_ap),
                eng.lower_val_access(eng.to_reg(num_idxs_reg)),
            ],
            outs=[eng.lower_ap(out_ap)],
            transpose=False,
            num_idxs=num_idxs,
            elem_size=elem_size,
            stride_bytes_256=stride_bytes_256,
            gen_mode=0,
            single_packet=False,
            queue_num=queue_num,
            sbuf_tokens_per_rank=0,
            sbuf_free_dim_per_rank=0,
            sbuf_free_dim_pad_per_rank=0,
            sbuf_byte_offset=0,
        )
    )


def _windows(n_chunks, width):
    """Compile-time window offsets; clamped ascending ladder."""
    top = width - WIN
    ws = []
    for c in range(n_chunks):
        w = int(round(16 * c - 5.6 * math.sqrt(c)))
        ws.append(min(top, max(0, w)))
    return ws


# ---------------------------------------------------------------- kernel A
def _build_kernel_a():
    if "A" in _cache:
        return _cache["A"]
    nc = bacc.Bacc("TRN2", target_bir_lowering=False, debug=False,
                   num_devices=N_CORES)
    xT = nc.dram_tensor("xT", [D, PC], BF16, kind="ExternalInput")
    w1 = nc.dram_tensor("w1", [D, H], BF16, kind="ExternalInput")
    b1 = nc.dram_tensor("b1", [H, 1], FP32, kind="ExternalInput")
    w2 = nc.dram_tensor("w2", [H, D], BF16, kind="ExternalInput")
    b2 = nc.dram_tensor("b2", [D, 1], FP32, kind="ExternalInput")
    xcT = nc.dram_tensor("xcT", [D, N_COLORS], BF16, kind="ExternalInput")
    w1c = nc.dram_tensor("w1c", [D, H], BF16, kind="ExternalInput")
    b1c = nc.dram_tensor("b1c", [H, 1], FP32, kind="ExternalInput")
    w2c = nc.dram_tensor("w2c", [H, D], BF16, kind="ExternalInput")
    b2c = nc.dram_tensor("b2c", [D, 1], FP32, kind="ExternalInput")
    msgT = nc.dram_tensor("msgT", [D, PC], BF16, kind="ExternalOutput")
    msgcT = nc.dram_tensor("msgcT", [D, N_COLORS], BF16, kind="ExternalOutput")

    S = 512
    with tile.TileContext(nc) as tc:
        with (
            tc.tile_pool(name="w", bufs=1) as wp,
            tc.tile_pool(name="act", bufs=3) as ap,
            tc.tile_pool(name="ps", bufs=2, space="PSUM") as pp,
        ):
            def mlp(xT_d, w1_d, b1_d, w2_d, b2_d, out_d, n_cols, tag):
                w1_t = wp.tile([D, H], BF16, tag=f"w1{tag}")
                b1_t = wp.tile([H, 1], FP32, tag=f"b1{tag}")
                w2_t = wp.tile([H, D], BF16, tag=f"w2{tag}")
                b2_t = wp.tile([D, 1], FP32, tag=f"b2{tag}")
                nc.sync.dma_start(out=w1_t[:], in_=w1_d[:])
                nc.sync.dma_start(out=b1_t[:], in_=b1_d[:])
                nc.sync.dma_start(out=w2_t[:], in_=w2_d[:])
                nc.sync.dma_start(out=b2_t[:], in_=b2_d[:])
                for s0 in range(0, n_cols, S):
                    s1 = min(s0 + S, n_cols)
                    w = s1 - s0
                    x_t = ap.tile([D, S], BF16, tag="x")
                    nc.sync.dma_start(out=x_t[:, :w], in_=xT_d[:, s0:s1])
                    h_ps = pp.tile([H, S], FP32, tag="h")
                    nc.tensor.matmul(out=h_ps[:, :w], lhsT=w1_t[:],
                                     rhs=x_t[:, :w], start=True, stop=True)
                    h_sb = ap.tile([H, S], BF16, tag="h_sb")
                    nc.scalar.activation(h_sb[:, :w], h_ps[:, :w], AF.Relu,
                                         bias=b1_t[:])
                    m_ps = pp.tile([D, S], FP32, tag="m")
                    nc.tensor.matmul(out=m_ps[:, :w], lhsT=w2_t[:],
                                     rhs=h_sb[:, :w], start=True, stop=True)
                    m_sb = ap.tile([D, S], BF16, tag="m_sb")
                    nc.scalar.activation(m_sb[:, :w], m_ps[:, :w], AF.Identity,
                                         bias=b2_t[:])
                    nc.sync.dma_start(out=out_d[:, s0:s1], in_=m_sb[:, :w])

            mlp(xT, w1, b1, w2, b2, msgT, PC, "v")
            mlp(xcT, w1c, b1c, w2c, b2c, msgcT, N_COLORS, "c")
    nc.compile()
    _cache["A"] = nc
    return nc


# ---------------------------------------------------------------- kernel B
def _build_kernel_b(CE, CEt, Ks):
    key = ("B", CE, CEt, Ks)
    if key in _cache:
        return _cache[key]
    CB = 2 * CE            # chunk columns per full bucket (even + odd)
    CBt = 2 * CEt
    IDXF = NB_FULL * CB * 8 + CBt * 8
    DLC = NB_FULL * CB + CBt

    nc = bacc.Bacc("TRN2", target_bir_lowering=False, debug=False,
                   num_devices=N_CORES, num_swdge_queues=4)
    table = nc.dram_tensor("table", [PAIRS, 2 * D], BF16, kind="ExternalInput")
    msgc = nc.dram_tensor("msgc", [N_COLORS, D], BF16, kind="ExternalInput")
    countT = nc.dram_tensor("countT", [N_COLORS, PC], BF16,
                            kind="ExternalInput")
    xT = nc.dram_tensor("xT", [D, PC], FP32, kind="ExternalInput")
    iota = nc.dram_tensor("iota", [128, WIN * CB], BF16, kind="ExternalInput")
    iota_tl = nc.dram_tensor("iota_tl", [128, WIN * CBt], BF16,
                             kind="ExternalInput")
    dstloc = nc.dram_tensor("dstloc", [128, DLC], BF16, kind="ExternalInput")
    gidx = nc.dram_tensor("gidx", [128, IDXF], I16, kind="ExternalInput")
    outT = nc.dram_tensor("outT", [D, PC], FP32, kind="ExternalOutput")

    wins_full = _windows(CE, BK)
    wins_tail = _windows(CEt, TAIL)

    with tile.TileContext(nc) as tc:
        with (
            tc.tile_pool(name="const", bufs=1) as cp,
            tc.tile_pool(name="gath", bufs=GP_BUFS) as gp,
            tc.tile_pool(name="idx", bufs=NB_FULL + 1) as ip,
            tc.tile_pool(name="oh", bufs=3) as ohp,
            tc.tile_pool(name="ct", bufs=4) as ctp,
            tc.tile_pool(name="ep", bufs=3) as ep,
            tc.tile_pool(name="ps", bufs=4, space="PSUM") as pp,
        ):
            # index tiles first so the first gathers launch immediately
            idx_es, idx_os = [], []
            for b in range(NB_FULL + 1):
                C1 = CE if b < NB_FULL else CEt
                ixoff = b * CB * 8
                idx_e = ip.tile([128, C1 * 8], I16, tag="ide")
                idx_o = ip.tile([128, C1 * 8], I16, tag="ido")
                nc.scalar.dma_start(out=idx_e[:],
                                    in_=gidx[:, ixoff:ixoff + C1 * 8])
                nc.scalar.dma_start(
                    out=idx_o[:],
                    in_=gidx[:, ixoff + C1 * 8:ixoff + 2 * C1 * 8])
                idx_es.append(idx_e)
                idx_os.append(idx_o)

            iota_t = cp.tile([128, WIN * CB], BF16, tag="iota")
            nc.scalar.dma_start(out=iota_t[:], in_=iota[:])
            iota_tt = cp.tile([128, WIN * CBt], BF16, tag="iota_tl")
            nc.scalar.dma_start(out=iota_tt[:], in_=iota_tl[:])
            dl_t = cp.tile([128, DLC], BF16, tag="dstloc")
            nc.scalar.dma_start(out=dl_t[:], in_=dstloc[:])
            mc0 = cp.tile([128, D], BF16, tag="msgc0")
            mc1 = cp.tile([128, D], BF16, tag="msgc1")
            nc.scalar.dma_start(out=mc0[:], in_=msgc[0:128, :])
            nc.scalar.dma_start(out=mc1[:], in_=msgc[128:256, :])

            for b in range(NB_FULL + 1):
                full = b < NB_FULL
                W = BK if full else TAIL
                C1 = CE if full else CEt
                wins = wins_full if full else wins_tail
                n0 = b * BK
                dloff = b * CB
                it_t = iota_t if full else iota_tt

                ge = gp.tile([128, C1, D], BF16, tag="ge")
                go = gp.tile([128, C1, D], BF16, tag="go")
                # Trailing slots >= K carry idx -1 on every core and are
                # skipped by the gather ucode.  Only trimmed for buckets
                # whose recycled pool buffer already holds finite values
                # (their zero one-hot columns then contribute exactly 0);
                # the first GP_BUFS buckets gather every slot.
                _dma_gather128(nc.gpsimd, ge[:], table[:, 0:D], idx_es[b][:],
                               C1 * 128, Ks[b][0], D, elem_step=2 * D,
                               queue_num=(2 * b) % 4)
                _dma_gather128(nc.gpsimd, go[:], table[:, D:2 * D],
                               idx_os[b][:], C1 * 128, Ks[b][1], D,
                               elem_step=2 * D, queue_num=(2 * b + 1) % 4)

                oh_t = ohp.tile([128, 2 * C1, WIN], BF16, tag="oh")
                nc.vector.tensor_tensor(
                    out=oh_t[:],
                    in0=it_t[:, :2 * C1 * WIN].rearrange(
                        "p (c n) -> p c n", n=WIN),
                    in1=dl_t[:, dloff:dloff + 2 * C1].to_broadcast(
                        [128, 2 * C1, WIN]),
                    op=mybir.AluOpType.is_equal)

                ct0 = ctp.tile([128, W], BF16, tag="ct0")
                ct1 = ctp.tile([128, W], BF16, tag="ct1")
                nc.sync.dma_start(out=ct0[:], in_=countT[0:128, n0:n0 + W])
                nc.sync.dma_start(out=ct1[:], in_=countT[128:256, n0:n0 + W])

                ps = pp.tile([D, W], FP32, tag="agg")
                nc.tensor.matmul(out=ps[:], lhsT=mc0[:], rhs=ct0[:],
                                 start=True, stop=False)
                nc.tensor.matmul(out=ps[:], lhsT=mc1[:], rhs=ct1[:],
                                 start=False, stop=False)
                for c in range(C1):      # even-parity chunks
                    w0 = wins[c]
                    nc.tensor.matmul(out=ps[:, w0:w0 + WIN],
                                     lhsT=ge[:, c, :],
                                     rhs=oh_t[:, c, :],
                                     start=False, stop=False)
                for c in range(C1):      # odd-parity chunks
                    w0 = wins[c]
                    nc.tensor.matmul(out=ps[:, w0:w0 + WIN],
                                     lhsT=go[:, c, :],
                                     rhs=oh_t[:, C1 + c, :],
                                     start=False, stop=(c == C1 - 1))

                xa = ep.tile([D, W], FP32, tag="xa")
                nc.sync.dma_start(out=xa[:], in_=xT[:, n0:n0 + W])
                s_t = ep.tile([D, W], FP32, tag="s")
                nc.vector.tensor_tensor(out=s_t[:], in0=ps[:], in1=xa[:],
                                        op=mybir.AluOpType.add)
                o_t = ep.tile([D, W], FP32, tag="o")
                nc.scalar.activation(o_t[:], s_t[:], AF.Relu)
                nc.sync.dma_start(out=outT[:, n0:n0 + W], in_=o_t[:])
    nc.compile()
    _cache[key] = nc
    return nc


# ------------------------------------------------------------- host logic
def _wrap_idx(flat):
    """dma_gather index layout: [16, n/16] wrapped, replicated to 128 rows."""
    n = flat.shape[0]
    assert n % 16 == 0
    w = flat.reshape(n // 16, 16).T.astype(np.int16)
    return np.tile(w, (8, 1))


def _assign_bucket(d_arr, pidx_arr, width, n_chunks, wins):
    """Greedy window assignment for one (core,bucket,parity) edge group.

    d_arr must be sorted ascending. Returns (slot_idx [C,128] int64,
    slot_dst [C,128] float32) or None if infeasible.
    """
    slot_idx = np.zeros((n_chunks, 128), np.int64)
    slot_dst = np.full((n_chunks, 128), PAD_DST, np.float32)
    if d_arr.shape[0] == 0:
        return slot_idx, slot_dst
    wins_a = np.asarray(wins)
    # lo[d]: first chunk whose window contains d; hi[d]: last such chunk
    ds = np.arange(width)
    lo_map = np.searchsorted(wins_a, ds - (WIN - 1), side="left")
    hi_map = np.searchsorted(wins_a, ds, side="right") - 1
    cnt = np.bincount(d_arr, minlength=width)
    fills = np.zeros(n_chunks, np.int64)
    pos = 0
    for d in range(width):
        need = int(cnt[d])
        if need == 0:
            continue
        c = int(lo_map[d])
        hi = int(hi_map[d])
        while need > 0:
            if c > hi or c >= n_chunks:
                return None
            take = min(need, 128 - int(fills[c]))
            if take > 0:
                f = int(fills[c])
                slot_idx[c, f:f + take] = pidx_arr[pos:pos + take]
                slot_dst[c, f:f + take] = d - wins[c]
                fills[c] += take
                pos += take
                need -= take
            if need > 0:
                c += 1
    return slot_idx, slot_dst


def _used_count(slot_dst):
    """Number of slots up to and including the last real edge (flat order)."""
    used = slot_dst.reshape(-1) != PAD_DST
    nz = np.nonzero(used)[0]
    return int(nz[-1]) + 1 if nz.size else 0


def _prep_vv(src, dst):
    """Bucket/sort/pad vv edges; returns CE, CEt, per-core gidx and dstloc."""
    src = src.astype(np.int64)
    dst = dst.astype(np.int64)
    core = dst // PC
    d_in_core = dst - core * PC
    bucket = np.minimum(d_in_core // BK, NB_FULL)
    d_local = d_in_core - bucket * BK
    parity = src & 1
    pidx = src >> 1

    key = ((core * (NB_FULL + 1) + bucket) * 2 + parity)
    order = np.lexsort((d_local, key))
    key_s = key[order]
    d_s = d_local[order]
    p_s = pidx[order]
    n_groups = N_CORES * (NB_FULL + 1) * 2
    counts = np.bincount(key_s, minlength=n_groups)
    starts = np.concatenate([[0], np.cumsum(counts)[:-1]])

    # global chunk counts
    cnt_full = counts.reshape(N_CORES, NB_FULL + 1, 2)
    CE = max(1, int(np.ceil(cnt_full[:, :NB_FULL, :].max() / 128)))
    CEt = max(1, int(np.ceil(cnt_full[:, NB_FULL, :].max() / 128)))

    for _ in range(4):
        wins_full = _windows(CE, BK)
        wins_tail = _windows(CEt, TAIL)
        res = [[None] * (2 * (NB_FULL + 1)) for _ in range(N_CORES)]
        ok = True
        for k in range(N_CORES):
            for b in range(NB_FULL + 1):
                fullb = b < NB_FULL
                width = BK if fullb else TAIL
                C1 = CE if fullb else CEt
                wins = wins_full if fullb else wins_tail
                for par in range(2):
                    gk = (k * (NB_FULL + 1) + b) * 2 + par
                    s0, c0 = starts[gk], counts[gk]
                    r = _assign_bucket(d_s[s0:s0 + c0], p_s[s0:s0 + c0],
                                       width, C1, wins)
                    if r is None:
                        ok = False
                        break
                    res[k][b * 2 + par] = r
                if not ok:
                    break
            if not ok:
                break
        if ok:
            break
        CE += 1
        CEt += 1
    else:
        raise RuntimeError("window assignment infeasible")

    # core-uniform trim counts: the gather ucode trims trailing -1 indices
    # and the decode reserves ring space from num_idxs_reg, so the trimmed
    # count must be identical on every core.
    Ks = []
    for b in range(NB_FULL + 1):
        kpair = []
        for par in range(2):
            n = max(_used_count(res[k][b * 2 + par][1])
                    for k in range(N_CORES))
            if not _TRIM_TAIL or b < GP_BUFS:
                n = (CE if b < NB_FULL else CEt) * 128
            kpair.append(max(n, 128))
        Ks.append(tuple(kpair))
    Ks = tuple(Ks)

    gidx, dstloc = [], []
    for k in range(N_CORES):
        parts_i, parts_d = [], []
        for b in range(NB_FULL + 1):
            ie, de = res[k][b * 2 + 0]
            io, do = res[k][b * 2 + 1]
            fe = ie.reshape(-1).copy()
            fo = io.reshape(-1).copy()
            fe[Ks[b][0]:] = -1
            fo[Ks[b][1]:] = -1
            parts_i.append(np.concatenate(
                [_wrap_idx(fe), _wrap_idx(fo)], axis=1))
            dl = np.concatenate([de, do], axis=0).T    # [128, 2*C1]
            parts_d.append(np.ascontiguousarray(dl))
        gidx.append(np.concatenate(parts_i, axis=1))
        dstloc.append(np.concatenate(parts_d, axis=1).astype(NPBF16))
    return CE, CEt, gidx, dstloc, Ks


def kernel(x_v, x_c, W1v, b1v, W2v, b2v, W1c, b1c, W2c, b2c,
           src_vv, dst_vv, src_vc, dst_vc):
    x_v = np.asarray(x_v, np.float32)
    x_c = np.asarray(x_c, np.float32)
    src_vv = np.asarray(src_vv, np.int32)
    dst_vv = np.asarray(dst_vv, np.int32)
    src_vc = np.asarray(src_vc, np.int32)
    dst_vc = np.asarray(dst_vc, np.int32)

    # ---------------- kernel A: message tables ----------------
    xT_full = np.zeros((D, NP), np.float32)
    xT_full[:, :N_NODES] = x_v.T
    a_common = {
        "w1": np.asarray(W1v, np.float32).astype(NPBF16),
        "b1": np.asarray(b1v, np.float32).reshape(H, 1),
        "w2": np.asarray(W2v, np.float32).astype(NPBF16),
        "b2": np.asarray(b2v, np.float32).reshape(D, 1),
        "xcT": np.ascontiguousarray(x_c.T).astype(NPBF16),
        "w1c": np.asarray(W1c, np.float32).astype(NPBF16),
        "b1c": np.asarray(b1c, np.float32).reshape(H, 1),
        "w2c": np.asarray(W2c, np.float32).astype(NPBF16),
        "b2c": np.asarray(b2c, np.float32).reshape(D, 1),
    }
    in_maps_a = []
    for k in range(N_CORES):
        m = dict(a_common)
        m["xT"] = np.ascontiguousarray(
            xT_full[:, k * PC:(k + 1) * PC]).astype(NPBF16)
        in_maps_a.append(m)
    nc_a = _build_kernel_a()
    res_a = _run(nc_a, in_maps_a, "A")

    msg = np.concatenate(
        [np.asarray(res_a[k]["msgT"]) for k in range(N_CORES)], axis=1).T
    msg_c = np.ascontiguousarray(np.asarray(res_a[0]["msgcT"]).T)  # [256,64]

    table = np.zeros((PAIRS, 2 * D), NPBF16)
    table[:NP // 2] = msg.reshape(NP // 2, 2 * D)

    # ---------------- host: index prep ----------------
    CE, CEt, gidx, dstloc, Ks = _prep_vv(src_vv, dst_vv)

    cnt = np.bincount(src_vc.astype(np.int64) * NP + dst_vc,
                      minlength=N_COLORS * NP).reshape(N_COLORS, NP)
    countT = cnt.astype(NPBF16)

    CB = 2 * CE
    CBt = 2 * CEt
    iota = np.tile(np.arange(WIN, dtype=np.float32),
                   (128, CB)).astype(NPBF16)
    iota_tl = np.tile(np.arange(WIN, dtype=np.float32),
                      (128, CBt)).astype(NPBF16)

    # ---------------- kernel B: gather + scatter + epilogue ----------------
    in_maps_b = []
    for k in range(N_CORES):
        in_maps_b.append({
            "table": table,
            "msgc": np.ascontiguousarray(msg_c.astype(NPBF16)),
            "countT": np.ascontiguousarray(countT[:, k * PC:(k + 1) * PC]),
            "xT": np.ascontiguousarray(xT_full[:, k * PC:(k + 1) * PC]),
            "iota": iota,
            "iota_tl": iota_tl,
            "dstloc": dstloc[k],
            "gidx": gidx[k],
        })
    nc_b = _build_kernel_b(CE, CEt, Ks)
    res_b = _run(nc_b, in_maps_b, "B")

    outT = np.concatenate(
        [np.asarray(res_b[k]["outT"]) for k in range(N_CORES)], axis=1)
    return np.ascontiguousarray(outT.T[:N_NODES]).astype(np.float32)


# revision 47
# speedup vs baseline: 1.2732x; 1.0136x over previous
"""GNN message-passing block on 8 Trainium2 NeuronCores.

Math: out[n] = relu(x_v[n] + agg_v[n] + agg_c[n])
    agg_v = segment_sum(MLPv(x_v)[src_vv], dst_vv)   (messages depend on src only)
    agg_c = Count @ MLPc(x_c)          (256 colors -> dense count matmul)

Design (v2, bf16):
  * Kernel A (node-sharded): computes the 50k-row message table in bf16.
  * Kernel B (dst-sharded): per-edge gather of bf16 pair-rows (256 B each,
    the dma_gather minimum element) + scatter-add via one-hot matmuls.
  * Edges are bucketed by 512-node dst range (one PSUM bank per bucket),
    split by src parity (a chunk's matmul reads the correct 64-column half
    of the gathered pair), and dst-sorted so each 128-edge chunk only
    covers a narrow 64-node window at a COMPILE-TIME offset (the window
    ladder w_c = 16c - 5.6*sqrt(c) is feasible w.h.p. for uniform edges;
    the host greedy verifies and bumps the chunk count on failure).
    The two full-width color-count matmuls run first with start=True so
    every PSUM element is initialized regardless of window coverage gaps.
  * The 13 per-bucket gathers rotate across all 4 SWDGE queues with deep
    buffering so descriptor generation and SDMA drain never go idle.
"""

import math

import numpy as np

import concourse.bacc as bacc
import concourse.mybir as mybir
import concourse.tile as tile
from concourse import ap_utils
from concourse._compat import exact_div
from concourse.bass import MemorySpace
from concourse.bass_utils import run_bass_kernel_spmd

FP32 = mybir.dt.float32
BF16 = mybir.dt.bfloat16
I16 = mybir.dt.int16
AF = mybir.ActivationFunctionType
NPBF16 = mybir.dt.np(BF16)

N_CORES = 8
N_NODES = 50000
N_COLORS = 256
D = 64
H = 128
NP = 50176              # nodes padded to 392 tiles of 128
PC = NP // N_CORES      # 6272 nodes per core
BK = 512                # bucket = one PSUM bank of fp32
NB_FULL = PC // BK      # 12 full buckets; tail bucket of 128 nodes
TAIL = PC - NB_FULL * BK
PAIRS = NP // 2 + 128   # bf16 pair-row table rows (padded)
WIN = 64                # one-hot window width
PAD_DST = 100.0

PROFILE = False
LAST_EXEC_NS = {}
_TRIM_TAIL = False
GP_BUFS = 6           # gather pool depth; buckets < GP_BUFS are untrimmed

_cache = {}


def _run(nc, in_maps, label):
    kwargs = {}
    if PROFILE:
        kwargs = dict(trace=True, trace_cores=[0])
    try:
        res = run_bass_kernel_spmd(nc, in_maps, list(range(N_CORES)), **kwargs)
    except Exception:
        if not kwargs:
            raise
        res = run_bass_kernel_spmd(nc, in_maps, list(range(N_CORES)))
    LAST_EXEC_NS[label] = res.exec_time_ns
    return res.results


def _dma_gather128(eng, out_ap, in_ap, idxs_ap, num_idxs, num_idxs_reg,
                   elem_size, elem_step, queue_num):
    """bass dma_gather for 128-byte elements.

    Identical to bass.GpSimd.dma_gather (non-transpose, DRAM source,
    immediate trigger) except the element only has to be a multiple of
    128 B; the row stride must still be a multiple of 256 B, which is the
    only granularity the descriptor ucode actually requires
    (stride_bytes_256).  The ucode's non-transpose path emits one plain
    CME descriptor of elem_size bytes per index, so 128 B is fine.
    """
    eng._assert_queue_num(queue_num)
    assert idxs_ap.dtype == mybir.dt.int16
    assert in_ap.dtype == out_ap.dtype
    assert in_ap.space == MemorySpace.DRAM
    assert idxs_ap.space == MemorySpace.SBUF
    assert out_ap.space == MemorySpace.SBUF
    elem_size_bytes = elem_size * mybir.dt.size(in_ap.dtype)
    assert elem_size_bytes % 128 == 0
    assert ap_utils.ap_is_contiguous(in_ap.ap[1:])
    assert ap_utils.ap_is_contiguous(out_ap.ap[1:])
    assert ap_utils.ap_is_contiguous(idxs_ap.ap[1:])
    assert in_ap.ap[-1][1] == out_ap.ap[-1][1] == elem_size
    assert out_ap.ap[0][1] * out_ap.ap[1][1] == num_idxs
    assert in_ap.ap[0][0] == elem_step
    stride_bytes_256 = exact_div(elem_step * mybir.dt.size(in_ap.dtype), 256)
    assert stride_bytes_256 < 256
    return eng.add_instruction(
        mybir.InstDMAGatherAnt(
            name=eng.bass.get_next_instruction_name(),
            ins=[
                *eng.lower_ap_dma(in_ap, for_custom_bir_dma=True),
                eng.lower_ap(idxs_ap),
                eng.lower_val_access(eng.to_reg(num_idxs_reg)),
            ],
            outs=[eng.lower_ap(out_ap)],
            transpose=False,
            num_idxs=num_idxs,
            elem_size=elem_size,
            stride_bytes_256=stride_bytes_256,
            gen_mode=0,
            single_packet=False,
            queue_num=queue_num,
            sbuf_tokens_per_rank=0,
            sbuf_free_dim_per_rank=0,
            sbuf_free_dim_pad_per_rank=0,
            sbuf_byte_offset=0,
        )
    )


def _windows(n_chunks, width):
    """Compile-time window offsets; clamped ascending ladder."""
    top = width - WIN
    ws = []
    for c in range(n_chunks):
        w = int(round(16 * c - 5.6 * math.sqrt(c)))
        ws.append(min(top, max(0, w)))
    return ws


# ---------------------------------------------------------------- kernel A
def _build_kernel_a():
    if "A" in _cache:
        return _cache["A"]
    nc = bacc.Bacc("TRN2", target_bir_lowering=False, debug=False,
                   num_devices=N_CORES)
    xT = nc.dram_tensor("xT", [D, PC], BF16, kind="ExternalInput")
    w1 = nc.dram_tensor("w1", [D, H], BF16, kind="ExternalInput")
    b1 = nc.dram_tensor("b1", [H, 1], FP32, kind="ExternalInput")
    w2 = nc.dram_tensor("w2", [H, D], BF16, kind="ExternalInput")
    b2 = nc.dram_tensor("b2", [D, 1], FP32, kind="ExternalInput")
    xcT = nc.dram_tensor("xcT", [D, N_COLORS], BF16, kind="ExternalInput")
    w1c = nc.dram_tensor("w1c", [D, H], BF16, kind="ExternalInput")
    b1c = nc.dram_tensor("b1c", [H, 1], FP32, kind="ExternalInput")
    w2c = nc.dram_tensor("w2c", [H, D], BF16, kind="ExternalInput")
    b2c = nc.dram_tensor("b2c", [D, 1], FP32, kind="ExternalInput")
    msgT = nc.dram_tensor("msgT", [D, PC], BF16, kind="ExternalOutput")
    msgcT = nc.dram_tensor("msgcT", [D, N_COLORS], BF16, kind="ExternalOutput")

    S = 512
    with tile.TileContext(nc) as tc:
        with (
            tc.tile_pool(name="w", bufs=1) as wp,
            tc.tile_pool(name="act", bufs=3) as ap,
            tc.tile_pool(name="ps", bufs=2, space="PSUM") as pp,
        ):
            def mlp(xT_d, w1_d, b1_d, w2_d, b2_d, out_d, n_cols, tag):
                w1_t = wp.tile([D, H], BF16, tag=f"w1{tag}")
                b1_t = wp.tile([H, 1], FP32, tag=f"b1{tag}")
                w2_t = wp.tile([H, D], BF16, tag=f"w2{tag}")
                b2_t = wp.tile([D, 1], FP32, tag=f"b2{tag}")
                nc.sync.dma_start(out=w1_t[:], in_=w1_d[:])
                nc.sync.dma_start(out=b1_t[:], in_=b1_d[:])
                nc.sync.dma_start(out=w2_t[:], in_=w2_d[:])
                nc.sync.dma_start(out=b2_t[:], in_=b2_d[:])
                for s0 in range(0, n_cols, S):
                    s1 = min(s0 + S, n_cols)
                    w = s1 - s0
                    x_t = ap.tile([D, S], BF16, tag="x")
                    nc.sync.dma_start(out=x_t[:, :w], in_=xT_d[:, s0:s1])
                    h_ps = pp.tile([H, S], FP32, tag="h")
                    nc.tensor.matmul(out=h_ps[:, :w], lhsT=w1_t[:],
                                     rhs=x_t[:, :w], start=True, stop=True)
                    h_sb = ap.tile([H, S], BF16, tag="h_sb")
                    nc.scalar.activation(h_sb[:, :w], h_ps[:, :w], AF.Relu,
                                         bias=b1_t[:])
                    m_ps = pp.tile([D, S], FP32, tag="m")
                    nc.tensor.matmul(out=m_ps[:, :w], lhsT=w2_t[:],
                                     rhs=h_sb[:, :w], start=True, stop=True)
                    m_sb = ap.tile([D, S], BF16, tag="m_sb")
                    # bias-add + bf16 cast on the otherwise idle vector
                    # engine so the scalar engine only runs the relu
                    nc.vector.tensor_scalar_add(m_sb[:, :w], m_ps[:, :w],
                                                b2_t[:])
                    nc.sync.dma_start(out=out_d[:, s0:s1], in_=m_sb[:, :w])

            mlp(xT, w1, b1, w2, b2, msgT, PC, "v")
            mlp(xcT, w1c, b1c, w2c, b2c, msgcT, N_COLORS, "c")
    nc.compile()
    _cache["A"] = nc
    return nc


# ---------------------------------------------------------------- kernel B
def _build_kernel_b(CE, CEt, Ks):
    key = ("B", CE, CEt, Ks)
    if key in _cache:
        return _cache[key]
    CB = 2 * CE            # chunk columns per full bucket (even + odd)
    CBt = 2 * CEt
    IDXF = NB_FULL * CB * 8 + CBt * 8
    DLC = NB_FULL * CB + CBt

    nc = bacc.Bacc("TRN2", target_bir_lowering=False, debug=False,
                   num_devices=N_CORES, num_swdge_queues=4)
    table = nc.dram_tensor("table", [PAIRS, 2 * D], BF16, kind="ExternalInput")
    msgc = nc.dram_tensor("msgc", [N_COLORS, D], BF16, kind="ExternalInput")
    countT = nc.dram_tensor("countT", [N_COLORS, PC], BF16,
                            kind="ExternalInput")
    xT = nc.dram_tensor("xT", [D, PC], FP32, kind="ExternalInput")
    iota = nc.dram_tensor("iota", [128, WIN * CB], BF16, kind="ExternalInput")
    iota_tl = nc.dram_tensor("iota_tl", [128, WIN * CBt], BF16,
                             kind="ExternalInput")
    dstloc = nc.dram_tensor("dstloc", [128, DLC], BF16, kind="ExternalInput")
    gidx = nc.dram_tensor("gidx", [128, IDXF], I16, kind="ExternalInput")
    outT = nc.dram_tensor("outT", [D, PC], FP32, kind="ExternalOutput")

    wins_full = _windows(CE, BK)
    wins_tail = _windows(CEt, TAIL)

    with tile.TileContext(nc) as tc:
        with (
            tc.tile_pool(name="const", bufs=1) as cp,
            tc.tile_pool(name="gath", bufs=GP_BUFS) as gp,
            tc.tile_pool(name="idx", bufs=NB_FULL + 1) as ip,
            tc.tile_pool(name="oh", bufs=3) as ohp,
            tc.tile_pool(name="ct", bufs=4) as ctp,
            tc.tile_pool(name="ep", bufs=3) as ep,
            tc.tile_pool(name="ps", bufs=4, space="PSUM") as pp,
        ):
            # index tiles first so the first gathers launch immediately
            idx_es, idx_os = [], []
            for b in range(NB_FULL + 1):
                C1 = CE if b < NB_FULL else CEt
                ixoff = b * CB * 8
                idx_e = ip.tile([128, C1 * 8], I16, tag="ide")
                idx_o = ip.tile([128, C1 * 8], I16, tag="ido")
                nc.scalar.dma_start(out=idx_e[:],
                                    in_=gidx[:, ixoff:ixoff + C1 * 8])
                nc.scalar.dma_start(
                    out=idx_o[:],
                    in_=gidx[:, ixoff + C1 * 8:ixoff + 2 * C1 * 8])
                idx_es.append(idx_e)
                idx_os.append(idx_o)

            iota_t = cp.tile([128, WIN * CB], BF16, tag="iota")
            nc.scalar.dma_start(out=iota_t[:], in_=iota[:])
            iota_tt = cp.tile([128, WIN * CBt], BF16, tag="iota_tl")
            nc.scalar.dma_start(out=iota_tt[:], in_=iota_tl[:])
            dl_t = cp.tile([128, DLC], BF16, tag="dstloc")
            nc.scalar.dma_start(out=dl_t[:], in_=dstloc[:])
            mc0 = cp.tile([128, D], BF16, tag="msgc0")
            mc1 = cp.tile([128, D], BF16, tag="msgc1")
            nc.scalar.dma_start(out=mc0[:], in_=msgc[0:128, :])
            nc.scalar.dma_start(out=mc1[:], in_=msgc[128:256, :])

            for b in range(NB_FULL + 1):
                full = b < NB_FULL
                W = BK if full else TAIL
                C1 = CE if full else CEt
                wins = wins_full if full else wins_tail
                n0 = b * BK
                dloff = b * CB
                it_t = iota_t if full else iota_tt

                ge = gp.tile([128, C1, D], BF16, tag="ge")
                go = gp.tile([128, C1, D], BF16, tag="go")
                # Trailing slots >= K carry idx -1 on every core and are
                # skipped by the gather ucode.  Only trimmed for buckets
                # whose recycled pool buffer already holds finite values
                # (their zero one-hot columns then contribute exactly 0);
                # the first GP_BUFS buckets gather every slot.
                _dma_gather128(nc.gpsimd, ge[:], table[:, 0:D], idx_es[b][:],
                               C1 * 128, Ks[b][0], D, elem_step=2 * D,
                               queue_num=(2 * b) % 4)
                _dma_gather128(nc.gpsimd, go[:], table[:, D:2 * D],
                               idx_os[b][:], C1 * 128, Ks[b][1], D,
                               elem_step=2 * D, queue_num=(2 * b + 1) % 4)

                oh_t = ohp.tile([128, 2 * C1, WIN], BF16, tag="oh")
                nc.vector.tensor_tensor(
                    out=oh_t[:],
                    in0=it_t[:, :2 * C1 * WIN].rearrange(
                        "p (c n) -> p c n", n=WIN),
                    in1=dl_t[:, dloff:dloff + 2 * C1].to_broadcast(
                        [128, 2 * C1, WIN]),
                    op=mybir.AluOpType.is_equal)

                ct0 = ctp.tile([128, W], BF16, tag="ct0")
                ct1 = ctp.tile([128, W], BF16, tag="ct1")
                nc.sync.dma_start(out=ct0[:], in_=countT[0:128, n0:n0 + W])
                nc.sync.dma_start(out=ct1[:], in_=countT[128:256, n0:n0 + W])

                ps = pp.tile([D, W], FP32, tag="agg")
                nc.tensor.matmul(out=ps[:], lhsT=mc0[:], rhs=ct0[:],
                                 start=True, stop=False)
                nc.tensor.matmul(out=ps[:], lhsT=mc1[:], rhs=ct1[:],
                                 start=False, stop=False)
                for c in range(C1):      # even-parity chunks
                    w0 = wins[c]
                    nc.tensor.matmul(out=ps[:, w0:w0 + WIN],
                                     lhsT=ge[:, c, :],
                                     rhs=oh_t[:, c, :],
                                     start=False, stop=False)
                for c in range(C1):      # odd-parity chunks
                    w0 = wins[c]
                    nc.tensor.matmul(out=ps[:, w0:w0 + WIN],
                                     lhsT=go[:, c, :],
                                     rhs=oh_t[:, C1 + c, :],
                                     start=False, stop=(c == C1 - 1))

                xa = ep.tile([D, W], FP32, tag="xa")
                nc.sync.dma_start(out=xa[:], in_=xT[:, n0:n0 + W])
                s_t = ep.tile([D, W], FP32, tag="s")
                nc.vector.tensor_tensor(out=s_t[:], in0=ps[:], in1=xa[:],
                                        op=mybir.AluOpType.add)
                o_t = ep.tile([D, W], FP32, tag="o")
                nc.scalar.activation(o_t[:], s_t[:], AF.Relu)
                nc.sync.dma_start(out=outT[:, n0:n0 + W], in_=o_t[:])
    nc.compile()
    _cache[key] = nc
    return nc


# ------------------------------------------------------------- host logic
def _wrap_idx(flat):
    """dma_gather index layout: [16, n/16] wrapped, replicated to 128 rows."""
    n = flat.shape[0]
    assert n % 16 == 0
    w = flat.reshape(n // 16, 16).T.astype(np.int16)
    return np.tile(w, (8, 1))


def _assign_bucket(d_arr, pidx_arr, width, n_chunks, wins):
    """Greedy window assignment for one (core,bucket,parity) edge group.

    d_arr must be sorted ascending. Returns (slot_idx [C,128] int64,
    slot_dst [C,128] float32) or None if infeasible.
    """
    slot_idx = np.zeros((n_chunks, 128), np.int64)
    slot_dst = np.full((n_chunks, 128), PAD_DST, np.float32)
    if d_arr.shape[0] == 0:
        return slot_idx, slot_dst
    wins_a = np.asarray(wins)
    # lo[d]: first chunk whose window contains d; hi[d]: last such chunk
    ds = np.arange(width)
    lo_map = np.searchsorted(wins_a, ds - (WIN - 1), side="left")
    hi_map = np.searchsorted(wins_a, ds, side="right") - 1
    cnt = np.bincount(d_arr, minlength=width)
    fills = np.zeros(n_chunks, np.int64)
    pos = 0
    for d in range(width):
        need = int(cnt[d])
        if need == 0:
            continue
        c = int(lo_map[d])
        hi = int(hi_map[d])
        while need > 0:
            if c > hi or c >= n_chunks:
                return None
            take = min(need, 128 - int(fills[c]))
            if take > 0:
                f = int(fills[c])
                slot_idx[c, f:f + take] = pidx_arr[pos:pos + take]
                slot_dst[c, f:f + take] = d - wins[c]
                fills[c] += take
                pos += take
                need -= take
            if need > 0:
                c += 1
    return slot_idx, slot_dst


def _used_count(slot_dst):
    """Number of slots up to and including the last real edge (flat order)."""
    used = slot_dst.reshape(-1) != PAD_DST
    nz = np.nonzero(used)[0]
    return int(nz[-1]) + 1 if nz.size else 0


def _prep_vv(src, dst):
    """Bucket/sort/pad vv edges; returns CE, CEt, per-core gidx and dstloc."""
    src = src.astype(np.int64)
    dst = dst.astype(np.int64)
    core = dst // PC
    d_in_core = dst - core * PC
    bucket = np.minimum(d_in_core // BK, NB_FULL)
    d_local = d_in_core - bucket * BK
    parity = src & 1
    pidx = src >> 1

    key = ((core * (NB_FULL + 1) + bucket) * 2 + parity)
    order = np.lexsort((d_local, key))
    key_s = key[order]
    d_s = d_local[order]
    p_s = pidx[order]
    n_groups = N_CORES * (NB_FULL + 1) * 2
    counts = np.bincount(key_s, minlength=n_groups)
    starts = np.concatenate([[0], np.cumsum(counts)[:-1]])

    # global chunk counts
    cnt_full = counts.reshape(N_CORES, NB_FULL + 1, 2)
    CE = max(1, int(np.ceil(cnt_full[:, :NB_FULL, :].max() / 128)))
    CEt = max(1, int(np.ceil(cnt_full[:, NB_FULL, :].max() / 128)))

    for _ in range(4):
        wins_full = _windows(CE, BK)
        wins_tail = _windows(CEt, TAIL)
        res = [[None] * (2 * (NB_FULL + 1)) for _ in range(N_CORES)]
        ok = True
        for k in range(N_CORES):
            for b in range(NB_FULL + 1):
                fullb = b < NB_FULL
                width = BK if fullb else TAIL
                C1 = CE if fullb else CEt
                wins = wins_full if fullb else wins_tail
                for par in range(2):
                    gk = (k * (NB_FULL + 1) + b) * 2 + par
                    s0, c0 = starts[gk], counts[gk]
                    r = _assign_bucket(d_s[s0:s0 + c0], p_s[s0:s0 + c0],
                                       width, C1, wins)
                    if r is None:
                        ok = False
                        break
                    res[k][b * 2 + par] = r
                if not ok:
                    break
            if not ok:
                break
        if ok:
            break
        CE += 1
        CEt += 1
    else:
        raise RuntimeError("window assignment infeasible")

    # core-uniform trim counts: the gather ucode trims trailing -1 indices
    # and the decode reserves ring space from num_idxs_reg, so the trimmed
    # count must be identical on every core.
    Ks = []
    for b in range(NB_FULL + 1):
        kpair = []
        for par in range(2):
            n = max(_used_count(res[k][b * 2 + par][1])
                    for k in range(N_CORES))
            if not _TRIM_TAIL or b < GP_BUFS:
                n = (CE if b < NB_FULL else CEt) * 128
            kpair.append(max(n, 128))
        Ks.append(tuple(kpair))
    Ks = tuple(Ks)

    gidx, dstloc = [], []
    for k in range(N_CORES):
        parts_i, parts_d = [], []
        for b in range(NB_FULL + 1):
            ie, de = res[k][b * 2 + 0]
            io, do = res[k][b * 2 + 1]
            fe = ie.reshape(-1).copy()
            fo = io.reshape(-1).copy()
            fe[Ks[b][0]:] = -1
            fo[Ks[b][1]:] = -1
            parts_i.append(np.concatenate(
                [_wrap_idx(fe), _wrap_idx(fo)], axis=1))
            dl = np.concatenate([de, do], axis=0).T    # [128, 2*C1]
            parts_d.append(np.ascontiguousarray(dl))
        gidx.append(np.concatenate(parts_i, axis=1))
        dstloc.append(np.concatenate(parts_d, axis=1).astype(NPBF16))
    return CE, CEt, gidx, dstloc, Ks


def kernel(x_v, x_c, W1v, b1v, W2v, b2v, W1c, b1c, W2c, b2c,
           src_vv, dst_vv, src_vc, dst_vc):
    x_v = np.asarray(x_v, np.float32)
    x_c = np.asarray(x_c, np.float32)
    src_vv = np.asarray(src_vv, np.int32)
    dst_vv = np.asarray(dst_vv, np.int32)
    src_vc = np.asarray(src_vc, np.int32)
    dst_vc = np.asarray(dst_vc, np.int32)

    # ---------------- kernel A: message tables ----------------
    xT_full = np.zeros((D, NP), np.float32)
    xT_full[:, :N_NODES] = x_v.T
    a_common = {
        "w1": np.asarray(W1v, np.float32).astype(NPBF16),
        "b1": np.asarray(b1v, np.float32).reshape(H, 1),
        "w2": np.asarray(W2v, np.float32).astype(NPBF16),
        "b2": np.asarray(b2v, np.float32).reshape(D, 1),
        "xcT": np.ascontiguousarray(x_c.T).astype(NPBF16),
        "w1c": np.asarray(W1c, np.float32).astype(NPBF16),
        "b1c": np.asarray(b1c, np.float32).reshape(H, 1),
        "w2c": np.asarray(W2c, np.float32).astype(NPBF16),
        "b2c": np.asarray(b2c, np.float32).reshape(D, 1),
    }
    in_maps_a = []
    for k in range(N_CORES):
        m = dict(a_common)
        m["xT"] = np.ascontiguousarray(
            xT_full[:, k * PC:(k + 1) * PC]).astype(NPBF16)
        in_maps_a.append(m)
    nc_a = _build_kernel_a()
    res_a = _run(nc_a, in_maps_a, "A")

    msg = np.concatenate(
        [np.asarray(res_a[k]["msgT"]) for k in range(N_CORES)], axis=1).T
    msg_c = np.ascontiguousarray(np.asarray(res_a[0]["msgcT"]).T)  # [256,64]

    table = np.zeros((PAIRS, 2 * D), NPBF16)
    table[:NP // 2] = msg.reshape(NP // 2, 2 * D)

    # ---------------- host: index prep ----------------
    CE, CEt, gidx, dstloc, Ks = _prep_vv(src_vv, dst_vv)

    cnt = np.bincount(src_vc.astype(np.int64) * NP + dst_vc,
                      minlength=N_COLORS * NP).reshape(N_COLORS, NP)
    countT = cnt.astype(NPBF16)

    CB = 2 * CE
    CBt = 2 * CEt
    iota = np.tile(np.arange(WIN, dtype=np.float32),
                   (128, CB)).astype(NPBF16)
    iota_tl = np.tile(np.arange(WIN, dtype=np.float32),
                      (128, CBt)).astype(NPBF16)

    # ---------------- kernel B: gather + scatter + epilogue ----------------
    in_maps_b = []
    for k in range(N_CORES):
        in_maps_b.append({
            "table": table,
            "msgc": np.ascontiguousarray(msg_c.astype(NPBF16)),
            "countT": np.ascontiguousarray(countT[:, k * PC:(k + 1) * PC]),
            "xT": np.ascontiguousarray(xT_full[:, k * PC:(k + 1) * PC]),
            "iota": iota,
            "iota_tl": iota_tl,
            "dstloc": dstloc[k],
            "gidx": gidx[k],
        })
    nc_b = _build_kernel_b(CE, CEt, Ks)
    res_b = _run(nc_b, in_maps_b, "B")

    outT = np.concatenate(
        [np.asarray(res_b[k]["outT"]) for k in range(N_CORES)], axis=1)
    return np.ascontiguousarray(outT.T[:N_NODES]).astype(np.float32)


# revision 49
# speedup vs baseline: 1.2772x; 1.0032x over previous
"""GNN message-passing block on 8 Trainium2 NeuronCores.

Math: out[n] = relu(x_v[n] + agg_v[n] + agg_c[n])
    agg_v = segment_sum(MLPv(x_v)[src_vv], dst_vv)   (messages depend on src only)
    agg_c = Count @ MLPc(x_c)          (256 colors -> dense count matmul)

Design (v2, bf16):
  * Kernel A (node-sharded): computes the 50k-row message table in bf16.
  * Kernel B (dst-sharded): per-edge gather of bf16 pair-rows (256 B each,
    the dma_gather minimum element) + scatter-add via one-hot matmuls.
  * Edges are bucketed by 512-node dst range (one PSUM bank per bucket),
    split by src parity (a chunk's matmul reads the correct 64-column half
    of the gathered pair), and dst-sorted so each 128-edge chunk only
    covers a narrow 64-node window at a COMPILE-TIME offset (the window
    ladder w_c = 16c - 5.6*sqrt(c) is feasible w.h.p. for uniform edges;
    the host greedy verifies and bumps the chunk count on failure).
    The two full-width color-count matmuls run first with start=True so
    every PSUM element is initialized regardless of window coverage gaps.
  * The 13 per-bucket gathers rotate across all 4 SWDGE queues with deep
    buffering so descriptor generation and SDMA drain never go idle.
"""

import math

import numpy as np

import concourse.bacc as bacc
import concourse.mybir as mybir
import concourse.tile as tile
from concourse import ap_utils
from concourse._compat import exact_div
from concourse.bass import MemorySpace
from concourse.bass_utils import run_bass_kernel_spmd

FP32 = mybir.dt.float32
BF16 = mybir.dt.bfloat16
I16 = mybir.dt.int16
AF = mybir.ActivationFunctionType
NPBF16 = mybir.dt.np(BF16)

N_CORES = 8
N_NODES = 50000
N_COLORS = 256
D = 64
H = 128
NP = 50176              # nodes padded to 392 tiles of 128
PC = NP // N_CORES      # 6272 nodes per core
BK = 512                # bucket = one PSUM bank of fp32
NB_FULL = PC // BK      # 12 full buckets; tail bucket of 128 nodes
TAIL = PC - NB_FULL * BK
PAIRS = NP // 2 + 128   # bf16 pair-row table rows (padded)
WIN = 64                # one-hot window width
PAD_DST = 100.0

PROFILE = False
LAST_EXEC_NS = {}
_TRIM_TAIL = False
_SINGLE_PACKET = False
GP_BUFS = 6           # gather pool depth; buckets < GP_BUFS are untrimmed

_cache = {}


def _run(nc, in_maps, label):
    kwargs = {}
    if PROFILE:
        kwargs = dict(trace=True, trace_cores=[0])
    try:
        res = run_bass_kernel_spmd(nc, in_maps, list(range(N_CORES)), **kwargs)
    except Exception:
        if not kwargs:
            raise
        res = run_bass_kernel_spmd(nc, in_maps, list(range(N_CORES)))
    LAST_EXEC_NS[label] = res.exec_time_ns
    return res.results


def _dma_gather128(eng, out_ap, in_ap, idxs_ap, num_idxs, num_idxs_reg,
                   elem_size, elem_step, queue_num):
    """bass dma_gather for 128-byte elements.

    Identical to bass.GpSimd.dma_gather (non-transpose, DRAM source,
    immediate trigger) except the element only has to be a multiple of
    128 B; the row stride must still be a multiple of 256 B, which is the
    only granularity the descriptor ucode actually requires
    (stride_bytes_256).  The ucode's non-transpose path emits one plain
    CME descriptor of elem_size bytes per index, so 128 B is fine.
    """
    eng._assert_queue_num(queue_num)
    assert idxs_ap.dtype == mybir.dt.int16
    assert in_ap.dtype == out_ap.dtype
    assert in_ap.space == MemorySpace.DRAM
    assert idxs_ap.space == MemorySpace.SBUF
    assert out_ap.space == MemorySpace.SBUF
    elem_size_bytes = elem_size * mybir.dt.size(in_ap.dtype)
    assert elem_size_bytes % 128 == 0
    assert ap_utils.ap_is_contiguous(in_ap.ap[1:])
    assert ap_utils.ap_is_contiguous(out_ap.ap[1:])
    assert ap_utils.ap_is_contiguous(idxs_ap.ap[1:])
    assert in_ap.ap[-1][1] == out_ap.ap[-1][1] == elem_size
    assert out_ap.ap[0][1] * out_ap.ap[1][1] == num_idxs
    assert in_ap.ap[0][0] == elem_step
    stride_bytes_256 = exact_div(elem_step * mybir.dt.size(in_ap.dtype), 256)
    assert stride_bytes_256 < 256
    return eng.add_instruction(
        mybir.InstDMAGatherAnt(
            name=eng.bass.get_next_instruction_name(),
            ins=[
                *eng.lower_ap_dma(in_ap, for_custom_bir_dma=True),
                eng.lower_ap(idxs_ap),
                eng.lower_val_access(eng.to_reg(num_idxs_reg)),
            ],
            outs=[eng.lower_ap(out_ap)],
            transpose=False,
            num_idxs=num_idxs,
            elem_size=elem_size,
            stride_bytes_256=stride_bytes_256,
            gen_mode=0,
            single_packet=_SINGLE_PACKET,
            queue_num=queue_num,
            sbuf_tokens_per_rank=0,
            sbuf_free_dim_per_rank=0,
            sbuf_free_dim_pad_per_rank=0,
            sbuf_byte_offset=0,
        )
    )


def _windows(n_chunks, width):
    """Compile-time window offsets; clamped ascending ladder."""
    top = width - WIN
    ws = []
    for c in range(n_chunks):
        w = int(round(16 * c - 5.6 * math.sqrt(c)))
        ws.append(min(top, max(0, w)))
    return ws


# ---------------------------------------------------------------- kernel A
def _build_kernel_a():
    if "A" in _cache:
        return _cache["A"]
    nc = bacc.Bacc("TRN2", target_bir_lowering=False, debug=False,
                   num_devices=N_CORES)
    xT = nc.dram_tensor("xT", [D, PC], BF16, kind="ExternalInput")
    w1 = nc.dram_tensor("w1", [D, H], BF16, kind="ExternalInput")
    b1 = nc.dram_tensor("b1", [H, 1], FP32, kind="ExternalInput")
    w2 = nc.dram_tensor("w2", [H, D], BF16, kind="ExternalInput")
    b2 = nc.dram_tensor("b2", [D, 1], FP32, kind="ExternalInput")
    xcT = nc.dram_tensor("xcT", [D, N_COLORS], BF16, kind="ExternalInput")
    w1c = nc.dram_tensor("w1c", [D, H], BF16, kind="ExternalInput")
    b1c = nc.dram_tensor("b1c", [H, 1], FP32, kind="ExternalInput")
    w2c = nc.dram_tensor("w2c", [H, D], BF16, kind="ExternalInput")
    b2c = nc.dram_tensor("b2c", [D, 1], FP32, kind="ExternalInput")
    msgT = nc.dram_tensor("msgT", [D, PC], BF16, kind="ExternalOutput")
    msgcT = nc.dram_tensor("msgcT", [D, N_COLORS], BF16, kind="ExternalOutput")

    S = 512
    with tile.TileContext(nc) as tc:
        with (
            tc.tile_pool(name="w", bufs=1) as wp,
            tc.tile_pool(name="act", bufs=3) as ap,
            tc.tile_pool(name="ps", bufs=2, space="PSUM") as pp,
        ):
            def mlp(xT_d, w1_d, b1_d, w2_d, b2_d, out_d, n_cols, tag):
                w1_t = wp.tile([D, H], BF16, tag=f"w1{tag}")
                b1_t = wp.tile([H, 1], FP32, tag=f"b1{tag}")
                w2_t = wp.tile([H, D], BF16, tag=f"w2{tag}")
                b2_t = wp.tile([D, 1], FP32, tag=f"b2{tag}")
                nc.sync.dma_start(out=w1_t[:], in_=w1_d[:])
                nc.sync.dma_start(out=b1_t[:], in_=b1_d[:])
                nc.sync.dma_start(out=w2_t[:], in_=w2_d[:])
                nc.sync.dma_start(out=b2_t[:], in_=b2_d[:])
                for s0 in range(0, n_cols, S):
                    s1 = min(s0 + S, n_cols)
                    w = s1 - s0
                    x_t = ap.tile([D, S], BF16, tag="x")
                    nc.sync.dma_start(out=x_t[:, :w], in_=xT_d[:, s0:s1])
                    h_ps = pp.tile([H, S], FP32, tag="h")
                    nc.tensor.matmul(out=h_ps[:, :w], lhsT=w1_t[:],
                                     rhs=x_t[:, :w], start=True, stop=True)
                    h_sb = ap.tile([H, S], BF16, tag="h_sb")
                    nc.scalar.activation(h_sb[:, :w], h_ps[:, :w], AF.Relu,
                                         bias=b1_t[:])
                    m_ps = pp.tile([D, S], FP32, tag="m")
                    nc.tensor.matmul(out=m_ps[:, :w], lhsT=w2_t[:],
                                     rhs=h_sb[:, :w], start=True, stop=True)
                    m_sb = ap.tile([D, S], BF16, tag="m_sb")
                    # bias-add + bf16 cast on the otherwise idle vector
                    # engine so the scalar engine only runs the relu
                    nc.vector.tensor_scalar_add(m_sb[:, :w], m_ps[:, :w],
                                                b2_t[:])
                    nc.sync.dma_start(out=out_d[:, s0:s1], in_=m_sb[:, :w])

            mlp(xT, w1, b1, w2, b2, msgT, PC, "v")
            mlp(xcT, w1c, b1c, w2c, b2c, msgcT, N_COLORS, "c")
    nc.compile()
    _cache["A"] = nc
    return nc


# ---------------------------------------------------------------- kernel B
def _build_kernel_b(CE, CEt, Ks):
    key = ("B", CE, CEt, Ks)
    if key in _cache:
        return _cache[key]
    CB = 2 * CE            # chunk columns per full bucket (even + odd)
    CBt = 2 * CEt
    IDXF = NB_FULL * CB * 8 + CBt * 8
    DLC = NB_FULL * CB + CBt

    nc = bacc.Bacc("TRN2", target_bir_lowering=False, debug=False,
                   num_devices=N_CORES, num_swdge_queues=4)
    table = nc.dram_tensor("table", [PAIRS, 2 * D], BF16, kind="ExternalInput")
    msgc = nc.dram_tensor("msgc", [N_COLORS, D], BF16, kind="ExternalInput")
    countT = nc.dram_tensor("countT", [N_COLORS, PC], BF16,
                            kind="ExternalInput")
    xT = nc.dram_tensor("xT", [D, PC], FP32, kind="ExternalInput")
    iota = nc.dram_tensor("iota", [128, WIN * CB], BF16, kind="ExternalInput")
    iota_tl = nc.dram_tensor("iota_tl", [128, WIN * CBt], BF16,
                             kind="ExternalInput")
    dstloc = nc.dram_tensor("dstloc", [128, DLC], BF16, kind="ExternalInput")
    gidx = nc.dram_tensor("gidx", [128, IDXF], I16, kind="ExternalInput")
    outT = nc.dram_tensor("outT", [D, PC], FP32, kind="ExternalOutput")

    wins_full = _windows(CE, BK)
    wins_tail = _windows(CEt, TAIL)

    with tile.TileContext(nc) as tc:
        with (
            tc.tile_pool(name="const", bufs=1) as cp,
            tc.tile_pool(name="gath", bufs=GP_BUFS) as gp,
            tc.tile_pool(name="idx", bufs=NB_FULL + 1) as ip,
            tc.tile_pool(name="oh", bufs=3) as ohp,
            tc.tile_pool(name="ct", bufs=4) as ctp,
            tc.tile_pool(name="ep", bufs=3) as ep,
            tc.tile_pool(name="ps", bufs=4, space="PSUM") as pp,
        ):
            # index tiles first so the first gathers launch immediately
            idx_es, idx_os = [], []
            for b in range(NB_FULL + 1):
                C1 = CE if b < NB_FULL else CEt
                ixoff = b * CB * 8
                idx_e = ip.tile([128, C1 * 8], I16, tag="ide")
                idx_o = ip.tile([128, C1 * 8], I16, tag="ido")
                nc.scalar.dma_start(out=idx_e[:],
                                    in_=gidx[:, ixoff:ixoff + C1 * 8])
                nc.scalar.dma_start(
                    out=idx_o[:],
                    in_=gidx[:, ixoff + C1 * 8:ixoff + 2 * C1 * 8])
                idx_es.append(idx_e)
                idx_os.append(idx_o)

            iota_t = cp.tile([128, WIN * CB], BF16, tag="iota")
            nc.scalar.dma_start(out=iota_t[:], in_=iota[:])
            iota_tt = cp.tile([128, WIN * CBt], BF16, tag="iota_tl")
            nc.scalar.dma_start(out=iota_tt[:], in_=iota_tl[:])
            dl_t = cp.tile([128, DLC], BF16, tag="dstloc")
            nc.scalar.dma_start(out=dl_t[:], in_=dstloc[:])
            mc0 = cp.tile([128, D], BF16, tag="msgc0")
            mc1 = cp.tile([128, D], BF16, tag="msgc1")
            nc.scalar.dma_start(out=mc0[:], in_=msgc[0:128, :])
            nc.scalar.dma_start(out=mc1[:], in_=msgc[128:256, :])

            for b in range(NB_FULL + 1):
                full = b < NB_FULL
                W = BK if full else TAIL
                C1 = CE if full else CEt
                wins = wins_full if full else wins_tail
                n0 = b * BK
                dloff = b * CB
                it_t = iota_t if full else iota_tt

                ge = gp.tile([128, C1, D], BF16, tag="ge")
                go = gp.tile([128, C1, D], BF16, tag="go")
                # Trailing slots >= K carry idx -1 on every core and are
                # skipped by the gather ucode.  Only trimmed for buckets
                # whose recycled pool buffer already holds finite values
                # (their zero one-hot columns then contribute exactly 0);
                # the first GP_BUFS buckets gather every slot.
                _dma_gather128(nc.gpsimd, ge[:], table[:, 0:D], idx_es[b][:],
                               C1 * 128, Ks[b][0], D, elem_step=2 * D,
                               queue_num=(2 * b) % 4)
                _dma_gather128(nc.gpsimd, go[:], table[:, D:2 * D],
                               idx_os[b][:], C1 * 128, Ks[b][1], D,
                               elem_step=2 * D, queue_num=(2 * b + 1) % 4)

                oh_t = ohp.tile([128, 2 * C1, WIN], BF16, tag="oh")
                nc.vector.tensor_tensor(
                    out=oh_t[:],
                    in0=it_t[:, :2 * C1 * WIN].rearrange(
                        "p (c n) -> p c n", n=WIN),
                    in1=dl_t[:, dloff:dloff + 2 * C1].to_broadcast(
                        [128, 2 * C1, WIN]),
                    op=mybir.AluOpType.is_equal)

                ct0 = ctp.tile([128, W], BF16, tag="ct0")
                ct1 = ctp.tile([128, W], BF16, tag="ct1")
                nc.sync.dma_start(out=ct0[:], in_=countT[0:128, n0:n0 + W])
                nc.sync.dma_start(out=ct1[:], in_=countT[128:256, n0:n0 + W])

                ps = pp.tile([D, W], FP32, tag="agg")
                nc.tensor.matmul(out=ps[:], lhsT=mc0[:], rhs=ct0[:],
                                 start=True, stop=False)
                nc.tensor.matmul(out=ps[:], lhsT=mc1[:], rhs=ct1[:],
                                 start=False, stop=False)
                for c in range(C1):      # even-parity chunks
                    w0 = wins[c]
                    nc.tensor.matmul(out=ps[:, w0:w0 + WIN],
                                     lhsT=ge[:, c, :],
                                     rhs=oh_t[:, c, :],
                                     start=False, stop=False)
                for c in range(C1):      # odd-parity chunks
                    w0 = wins[c]
                    nc.tensor.matmul(out=ps[:, w0:w0 + WIN],
                                     lhsT=go[:, c, :],
                                     rhs=oh_t[:, C1 + c, :],
                                     start=False, stop=(c == C1 - 1))

                xa = ep.tile([D, W], FP32, tag="xa")
                nc.sync.dma_start(out=xa[:], in_=xT[:, n0:n0 + W])
                s_t = ep.tile([D, W], FP32, tag="s")
                nc.vector.tensor_tensor(out=s_t[:], in0=ps[:], in1=xa[:],
                                        op=mybir.AluOpType.add)
                o_t = ep.tile([D, W], FP32, tag="o")
                nc.scalar.activation(o_t[:], s_t[:], AF.Relu)
                nc.sync.dma_start(out=outT[:, n0:n0 + W], in_=o_t[:])
    nc.compile()
    _cache[key] = nc
    return nc


# ------------------------------------------------------------- host logic
def _wrap_idx(flat):
    """dma_gather index layout: [16, n/16] wrapped, replicated to 128 rows."""
    n = flat.shape[0]
    assert n % 16 == 0
    w = flat.reshape(n // 16, 16).T.astype(np.int16)
    return np.tile(w, (8, 1))


def _assign_bucket(d_arr, pidx_arr, width, n_chunks, wins):
    """Greedy window assignment for one (core,bucket,parity) edge group.

    d_arr must be sorted ascending. Returns (slot_idx [C,128] int64,
    slot_dst [C,128] float32) or None if infeasible.
    """
    slot_idx = np.zeros((n_chunks, 128), np.int64)
    slot_dst = np.full((n_chunks, 128), PAD_DST, np.float32)
    if d_arr.shape[0] == 0:
        return slot_idx, slot_dst
    wins_a = np.asarray(wins)
    # lo[d]: first chunk whose window contains d; hi[d]: last such chunk
    ds = np.arange(width)
    lo_map = np.searchsorted(wins_a, ds - (WIN - 1), side="left")
    hi_map = np.searchsorted(wins_a, ds, side="right") - 1
    cnt = np.bincount(d_arr, minlength=width)
    fills = np.zeros(n_chunks, np.int64)
    pos = 0
    for d in range(width):
        need = int(cnt[d])
        if need == 0:
            continue
        c = int(lo_map[d])
        hi = int(hi_map[d])
        while need > 0:
            if c > hi or c >= n_chunks:
                return None
            take = min(need, 128 - int(fills[c]))
            if take > 0:
                f = int(fills[c])
                slot_idx[c, f:f + take] = pidx_arr[pos:pos + take]
                slot_dst[c, f:f + take] = d - wins[c]
                fills[c] += take
                pos += take
                need -= take
            if need > 0:
                c += 1
    return slot_idx, slot_dst


def _used_count(slot_dst):
    """Number of slots up to and including the last real edge (flat order)."""
    used = slot_dst.reshape(-1) != PAD_DST
    nz = np.nonzero(used)[0]
    return int(nz[-1]) + 1 if nz.size else 0


def _prep_vv(src, dst):
    """Bucket/sort/pad vv edges; returns CE, CEt, per-core gidx and dstloc."""
    src = src.astype(np.int64)
    dst = dst.astype(np.int64)
    core = dst // PC
    d_in_core = dst - core * PC
    bucket = np.minimum(d_in_core // BK, NB_FULL)
    d_local = d_in_core - bucket * BK
    parity = src & 1
    pidx = src >> 1

    key = ((core * (NB_FULL + 1) + bucket) * 2 + parity)
    order = np.lexsort((d_local, key))
    key_s = key[order]
    d_s = d_local[order]
    p_s = pidx[order]
    n_groups = N_CORES * (NB_FULL + 1) * 2
    counts = np.bincount(key_s, minlength=n_groups)
    starts = np.concatenate([[0], np.cumsum(counts)[:-1]])

    # global chunk counts
    cnt_full = counts.reshape(N_CORES, NB_FULL + 1, 2)
    CE = max(1, int(np.ceil(cnt_full[:, :NB_FULL, :].max() / 128)))
    CEt = max(1, int(np.ceil(cnt_full[:, NB_FULL, :].max() / 128)))

    for _ in range(4):
        wins_full = _windows(CE, BK)
        wins_tail = _windows(CEt, TAIL)
        res = [[None] * (2 * (NB_FULL + 1)) for _ in range(N_CORES)]
        ok = True
        for k in range(N_CORES):
            for b in range(NB_FULL + 1):
                fullb = b < NB_FULL
                width = BK if fullb else TAIL
                C1 = CE if fullb else CEt
                wins = wins_full if fullb else wins_tail
                for par in range(2):
                    gk = (k * (NB_FULL + 1) + b) * 2 + par
                    s0, c0 = starts[gk], counts[gk]
                    r = _assign_bucket(d_s[s0:s0 + c0], p_s[s0:s0 + c0],
                                       width, C1, wins)
                    if r is None:
                        ok = False
                        break
                    res[k][b * 2 + par] = r
                if not ok:
                    break
            if not ok:
                break
        if ok:
            break
        CE += 1
        CEt += 1
    else:
        raise RuntimeError("window assignment infeasible")

    # core-uniform trim counts: the gather ucode trims trailing -1 indices
    # and the decode reserves ring space from num_idxs_reg, so the trimmed
    # count must be identical on every core.
    Ks = []
    for b in range(NB_FULL + 1):
        kpair = []
        for par in range(2):
            n = max(_used_count(res[k][b * 2 + par][1])
                    for k in range(N_CORES))
            if not _TRIM_TAIL or b < GP_BUFS:
                n = (CE if b < NB_FULL else CEt) * 128
            kpair.append(max(n, 128))
        Ks.append(tuple(kpair))
    Ks = tuple(Ks)

    gidx, dstloc = [], []
    for k in range(N_CORES):
        parts_i, parts_d = [], []
        for b in range(NB_FULL + 1):
            ie, de = res[k][b * 2 + 0]
            io, do = res[k][b * 2 + 1]
            fe = ie.reshape(-1).copy()
            fo = io.reshape(-1).copy()
            fe[Ks[b][0]:] = -1
            fo[Ks[b][1]:] = -1
            parts_i.append(np.concatenate(
                [_wrap_idx(fe), _wrap_idx(fo)], axis=1))
            dl = np.concatenate([de, do], axis=0).T    # [128, 2*C1]
            parts_d.append(np.ascontiguousarray(dl))
        gidx.append(np.concatenate(parts_i, axis=1))
        dstloc.append(np.concatenate(parts_d, axis=1).astype(NPBF16))
    return CE, CEt, gidx, dstloc, Ks


def kernel(x_v, x_c, W1v, b1v, W2v, b2v, W1c, b1c, W2c, b2c,
           src_vv, dst_vv, src_vc, dst_vc):
    x_v = np.asarray(x_v, np.float32)
    x_c = np.asarray(x_c, np.float32)
    src_vv = np.asarray(src_vv, np.int32)
    dst_vv = np.asarray(dst_vv, np.int32)
    src_vc = np.asarray(src_vc, np.int32)
    dst_vc = np.asarray(dst_vc, np.int32)

    # ---------------- kernel A: message tables ----------------
    xT_full = np.zeros((D, NP), np.float32)
    xT_full[:, :N_NODES] = x_v.T
    a_common = {
        "w1": np.asarray(W1v, np.float32).astype(NPBF16),
        "b1": np.asarray(b1v, np.float32).reshape(H, 1),
        "w2": np.asarray(W2v, np.float32).astype(NPBF16),
        "b2": np.asarray(b2v, np.float32).reshape(D, 1),
        "xcT": np.ascontiguousarray(x_c.T).astype(NPBF16),
        "w1c": np.asarray(W1c, np.float32).astype(NPBF16),
        "b1c": np.asarray(b1c, np.float32).reshape(H, 1),
        "w2c": np.asarray(W2c, np.float32).astype(NPBF16),
        "b2c": np.asarray(b2c, np.float32).reshape(D, 1),
    }
    in_maps_a = []
    for k in range(N_CORES):
        m = dict(a_common)
        m["xT"] = np.ascontiguousarray(
            xT_full[:, k * PC:(k + 1) * PC]).astype(NPBF16)
        in_maps_a.append(m)
    nc_a = _build_kernel_a()
    res_a = _run(nc_a, in_maps_a, "A")

    msg = np.concatenate(
        [np.asarray(res_a[k]["msgT"]) for k in range(N_CORES)], axis=1).T
    msg_c = np.ascontiguousarray(np.asarray(res_a[0]["msgcT"]).T)  # [256,64]

    table = np.zeros((PAIRS, 2 * D), NPBF16)
    table[:NP // 2] = msg.reshape(NP // 2, 2 * D)

    # ---------------- host: index prep ----------------
    CE, CEt, gidx, dstloc, Ks = _prep_vv(src_vv, dst_vv)

    cnt = np.bincount(src_vc.astype(np.int64) * NP + dst_vc,
                      minlength=N_COLORS * NP).reshape(N_COLORS, NP)
    countT = cnt.astype(NPBF16)

    CB = 2 * CE
    CBt = 2 * CEt
    iota = np.tile(np.arange(WIN, dtype=np.float32),
                   (128, CB)).astype(NPBF16)
    iota_tl = np.tile(np.arange(WIN, dtype=np.float32),
                      (128, CBt)).astype(NPBF16)

    # ---------------- kernel B: gather + scatter + epilogue ----------------
    in_maps_b = []
    for k in range(N_CORES):
        in_maps_b.append({
            "table": table,
            "msgc": np.ascontiguousarray(msg_c.astype(NPBF16)),
            "countT": np.ascontiguousarray(countT[:, k * PC:(k + 1) * PC]),
            "xT": np.ascontiguousarray(xT_full[:, k * PC:(k + 1) * PC]),
            "iota": iota,
            "iota_tl": iota_tl,
            "dstloc": dstloc[k],
            "gidx": gidx[k],
        })
    nc_b = _build_kernel_b(CE, CEt, Ks)
    res_b = _run(nc_b, in_maps_b, "B")

    outT = np.concatenate(
        [np.asarray(res_b[k]["outT"]) for k in range(N_CORES)], axis=1)
    return np.ascontiguousarray(outT.T[:N_NODES]).astype(np.float32)
